# revision 27
# baseline (speedup 1.0000x reference)
"""GroupedQueryAttention Trainium2 kernel (8-core SPMD, bf16 datapath).

Problem: B=2, S=2048, D=2048, 32 Q heads, 8 KV groups, head_dim=64.
  q = xq @ Wq + bq; k = xk @ Wk + bk; v = xv @ Wv + bv
  logits = q . k / sqrt(512), causal softmax, out = (attn @ v) @ Wo + bo

Sharding: one batch x two KV groups per core (2 batches x 4 group-pairs = 8).
Each core computes its 8 Q heads' attention and a partial output projection
(rows of Wo for its 512 channels); the host sums the 4 partials per batch and
adds the bv/bo corrections (bv passes through softmax linearly since attention
weights sum to 1, so bv_expand @ Wo + bo is exact).

Perf notes vs the fp32 version:
- All matmul operands are bf16 (host casts inputs/weights; PSUM accumulation
  stays fp32): 1 PE cycle/row at any width vs fp32's two-pass LOW_HIGH mode.
- Softmax denominators are folded into the attn@v matmul via a ones column
  appended to each head-group's V block (M=65), killing the dedicated
  ones-vector matmul streams.
- V is projected transposed (weights stationary, x streaming) then flipped
  with PE transpose ops - much cheaper than streaming 128-wide W with x tiles
  as stationary weights.
- The two logit halves of a key block land in one 2-bank PSUM tile so a single
  wide activation does exp for both (fewer Act fixed overheads).
- Reciprocal uses the fast-approx DVE op (f32), downcast to bf16 on the Act
  engine (single-partition DVE ops are lane-serial and slow).
- Input/output DMA spread across scalar/gpsimd/sync queues (~95GB/s each).
- Wo projection of superblock i is emitted after the first head-pair of
  superblock i+1 so its matmuls never head-block the PE queue.
"""

import math
import numpy as np
import ml_dtypes

import concourse.bass as bass
import concourse.mybir as mybir
from concourse import tile
from concourse.bass_utils import run_bass_kernel_spmd
from concourse.vector_clock import ScopedClock

F32 = mybir.dt.float32
BF16 = mybir.dt.bfloat16
NPBF16 = ml_dtypes.bfloat16
B, S, D = 2, 2048, 2048
NKV, HPG, HD = 8, 4, 64
DIMK = 512                 # k/v projection width; also the softmax scale base
CPC = 512                  # q channels per core (2 groups * 4 heads * 64)
KC = D // 128              # 16 k-chunks
MSB = S // 512             # 4 m-superblocks
NB = S // 128              # 16 n-blocks
VST = 130                  # v_sb per-block stride: 64 v_a | 1 | 64 v_b | 1
INV_SQRT_DIMK = 1.0 / math.sqrt(float(DIMK))


# ---------------------------------------------------------------------------
# TileContext tail-drain patch: the bundled neuronxcc walrus rejects
# instructions carrying more than ~2 sync waits ("Too many sync wait
# commands"). Spread the kernel-tail waits over single-wait nops.
def _patched_drain_and_barrier(self, tick_clock, wait_clock):
    nc = self.nc
    collector = nc.sync.nop(nofuse=True)
    wait_clock.add_sem_waits(
        collector.ins, ScopedClock({None: tick_clock.global_clock})
    )
    si = collector.ins.sync_info
    waits = list(si.on_wait) if si is not None and si.on_wait else []
    if waits:
        collector.ins.sync_info = mybir.SyncInfo(
            on_wait=[waits[0]], on_update=list(si.on_update or [])
        )
        for w in waits[1:]:
            extra = nc.sync.nop(nofuse=True)
            extra.ins.sync_info = mybir.SyncInfo(on_wait=[w], on_update=[])
    nc.sync.drain()
    nc.all_engine_barrier()
    assert self.sems is not None
    popped = nc._tile_sem_poison_stack.pop()
    assert popped is self._sem_poison
    nc.clear_and_free_semaphores(list(self.sems.allocated().values()))
    nc.all_engine_barrier()


tile.TileContext._drain_and_barrier = _patched_drain_and_barrier


_MAXW = 1
_NOPID = [0]


def split_excess_waits(nc):
    """Walrus here encodes at most ~1-2 sync waits per instruction; move the
    excess onto preceding same-engine nops (engine order preserves timing)."""
    for f in nc.m.functions:
        for bb in f.blocks:
            out_list = []
            changed = False
            for inst in bb.instructions:
                si = getattr(inst, "sync_info", None)
                waits = list(si.on_wait) if si is not None and si.on_wait else []
                if len(waits) > _MAXW:
                    changed = True
                    for w in waits[:-_MAXW]:
                        _NOPID[0] += 1
                        nop = mybir.InstNoOp(
                            name=f"waitnop-{_NOPID[0]}", ins=[], outs=[],
                            engine=inst.engine,
                        )
                        nop.sync_info = mybir.SyncInfo(on_wait=[w], on_update=[])
                        out_list.append(nop)
                    inst.sync_info = mybir.SyncInfo(
                        on_wait=waits[-_MAXW:], on_update=list(si.on_update or [])
                    )
                out_list.append(inst)
            if changed:
                bb.instructions[:] = out_list
# ---------------------------------------------------------------------------


def build_bass():
    nc = bass.Bass()
    # x inputs arrive pre-blocked [kc, nsb, 128, 512] so every chunk DMA is
    # one fully linear 128KB read (strided 1KB bf16 lines run at half rate).
    xqT = nc.dram_tensor("xqT", [KC, 4, 128, 512], BF16, kind="ExternalInput")
    xkT = nc.dram_tensor("xkT", [KC, 4, 128, 512], BF16, kind="ExternalInput")
    xvT = nc.dram_tensor("xvT", [KC, 4, 128, 512], BF16, kind="ExternalInput")
    wq = nc.dram_tensor("wq", [D, CPC], BF16, kind="ExternalInput")
    wk = nc.dram_tensor("wk", [D, 128], BF16, kind="ExternalInput")
    wv = nc.dram_tensor("wv", [D, 128], BF16, kind="ExternalInput")
    wo = nc.dram_tensor("wo", [CPC, D], BF16, kind="ExternalInput")
    bq = nc.dram_tensor("bq", [CPC, 1], F32, kind="ExternalInput")
    bk = nc.dram_tensor("bk", [128, 1], F32, kind="ExternalInput")
    trimask = nc.dram_tensor("trimask", [128, 128], BF16, kind="ExternalInput")
    ident = nc.dram_tensor("ident", [128, 128], BF16, kind="ExternalInput")
    out = nc.dram_tensor("out", [S, D], BF16, kind="ExternalOutput")

    from contextlib import ExitStack
    with tile.TileContext(nc) as tc, ExitStack() as ctx:
        build_body(ctx, tc, xqT, xkT, xvT, wq, wk, wv, wo, bq, bk,
                   trimask, ident, out)
    split_excess_waits(nc)
    return nc


def build_body(ctx, tc, xqT, xkT, xvT, wq, wk, wv, wo, bq, bk,
               trimask, ident, out):
    nc = tc.nc
    Exp = mybir.ActivationFunctionType.Exp
    Ident = mybir.ActivationFunctionType.Identity
    Copy = mybir.ActivationFunctionType.Copy

    const = ctx.enter_context(tc.tile_pool(name="const", bufs=1))
    wq_sb = const.tile([128, KC * CPC], BF16, tag="wq")      # [128, 8192]
    wk_sb = const.tile([128, KC * 128], BF16, tag="wk")      # [128, 2048]
    wv_sb = const.tile([128, KC * 128], BF16, tag="wv")      # [128, 2048]
    wo_sb = const.tile([128, 4 * D], BF16, tag="wo")         # [128, 8192]
    kT_sb = const.tile([128, S], BF16, tag="kT")             # [128, 2048]
    v_sb = const.tile([128, NB * VST], BF16, tag="v")        # [128, 2080]
    qT_sb = const.tile([128, 4 * S], BF16, tag="qT")         # [128, 8192]
    bq_sb = const.tile([128, 4], F32, tag="bq")
    bk_sb = const.tile([128, 1], F32, tag="bk")
    mask_sb = const.tile([128, 128], BF16, tag="mask")
    ident_sb = const.tile([128, 128], BF16, tag="ident")
    ones_row = const.tile([1, 64], BF16, tag="ones_row")

    # Weight / bias / mask loads: each weight rides its consumer stream's
    # queue just ahead of the x chunks (xk->gpsimd, xv->sync, xq->scalar);
    # wo is loaded late (emitted after the projection loop) on sync.
    nc.gpsimd.dma_start(
        wk_sb[:].rearrange("p (kc c) -> p kc c", kc=KC),
        wk.rearrange("(kc p) c -> p kc c", p=128),
    )
    nc.sync.dma_start(
        wv_sb[:].rearrange("p (kc c) -> p kc c", kc=KC),
        wv.rearrange("(kc p) c -> p kc c", p=128),
    )
    nc.scalar.dma_start(
        wq_sb[:].rearrange("p (kc c) -> p kc c", kc=KC),
        wq.rearrange("(kc p) c -> p kc c", p=128),
    )
    nc.scalar.dma_start(
        bq_sb[:].rearrange("p (cb o) -> p cb o", cb=4),
        bq.rearrange("(cb p) o -> p cb o", p=128),
    )
    nc.scalar.dma_start(bk_sb[:], bk[:])
    nc.scalar.dma_start(mask_sb[:], trimask[:])
    nc.scalar.dma_start(ident_sb[:], ident[:])
    nc.vector.memset(v_sb[:], 1.0)   # ones columns at 64/129 of each block
    nc.vector.memset(ones_row[:], 1.0)

    # ---------------- Phase 1-3: projections ----------------
    with tc.tile_pool(name="proj_psum", bufs=6, space="PSUM") as proj_psum, \
         tc.tile_pool(name="tp_psum", bufs=2, space="PSUM") as tp_psum, \
         tc.tile_pool(name="xin", bufs=5) as xin_pool, \
         tc.tile_pool(name="xvin", bufs=5) as xvin_pool, \
         tc.tile_pool(name="vt", bufs=2) as vt_pool:

        # K/V/Q projections interleaved per 512-token superblock so that all
        # three x input streams (gpsimd/sync/scalar queues) run concurrently.
        for nsb in range(4):
            # K: kT[c=128, n] accumulated over k-chunks, bias bk.
            ps = proj_psum.tile([128, 512], F32, tag="ps")
            for kc in range(KC):
                xk_t = xin_pool.tile([128, 512], BF16, tag="xk")
                nc.gpsimd.dma_start(
                    xk_t[:], xkT[kc, nsb]
                )
                nc.tensor.matmul(
                    ps[:], wk_sb[:, kc * 128:(kc + 1) * 128], xk_t[:],
                    start=(kc == 0), stop=(kc == KC - 1),
                )
            nc.scalar.activation(
                kT_sb[:, nsb * 512:(nsb + 1) * 512], ps[:], Ident, bias=bk_sb[:]
            )

            # V: projected transposed (vT[c, n]) with wv stationary, then PE
            # transposes into v natural layout with interleaved ones columns.
            ps = proj_psum.tile([128, 512], F32, tag="ps")
            for kc in range(KC):
                xv_t = xvin_pool.tile([128, 512], BF16, tag="xv")
                nc.sync.dma_start(
                    xv_t[:], xvT[kc, nsb]
                )
                nc.tensor.matmul(
                    ps[:], wv_sb[:, kc * 128:(kc + 1) * 128], xv_t[:],
                    start=(kc == 0), stop=(kc == KC - 1),
                )
            vT_t = vt_pool.tile([128, 512], BF16, tag="vt")
            nc.vector.tensor_copy(vT_t[:], ps[:])
            for t in range(4):
                j = nsb * 4 + t
                tp = tp_psum.tile([128, 128], BF16, tag="tp")
                nc.tensor.transpose(tp[:], vT_t[:, t * 128:(t + 1) * 128],
                                    ident_sb[:])
                nc.vector.tensor_copy(v_sb[:, j * VST: j * VST + 64],
                                      tp[:, 0:64])
                nc.vector.tensor_copy(v_sb[:, j * VST + 65: j * VST + 129],
                                      tp[:, 64:128])

            # Q: qT[c, m], layout [msb][cb][512], bias bq (msb == nsb).
            msb = nsb
            pss = [proj_psum.tile([128, 512], F32, tag="ps", name=f"psq{cb}")
                   for cb in range(4)]
            for kc in range(KC):
                xq_t = xin_pool.tile([128, 512], BF16, tag="xq")
                nc.scalar.dma_start(
                    xq_t[:], xqT[kc, msb]
                )
                for cb in range(4):
                    nc.tensor.matmul(
                        pss[cb][:],
                        wq_sb[:, kc * CPC + cb * 128: kc * CPC + (cb + 1) * 128],
                        xq_t[:],
                        start=(kc == 0), stop=(kc == KC - 1),
                    )
            for cb in range(4):
                nc.scalar.activation(
                    qT_sb[:, msb * 2048 + cb * 512: msb * 2048 + (cb + 1) * 512],
                    pss[cb][:], Ident, bias=bq_sb[:, cb:cb + 1],
                )

    # wo lands during early attention; first consumer is ~2/3 in.
    nc.sync.dma_start(
        wo_sb[:].rearrange("p (cb d) -> p cb d", cb=4),
        wo.rearrange("(cb p) d -> p cb d", p=128),
    )

    # ---------------- Phase 4: attention + output projection ----------------
    with tc.tile_pool(name="lt_psum", bufs=2, space="PSUM") as lt_psum, \
         tc.tile_pool(name="acc_psum", bufs=1, space="PSUM") as acc_psum, \
         tc.tile_pool(name="bps_psum", bufs=1, space="PSUM") as bps_psum, \
         tc.tile_pool(name="wo_psum", bufs=1, space="PSUM") as wo_psum, \
         tc.tile_pool(name="pt", bufs=3) as pt_pool, \
         tc.tile_pool(name="outT", bufs=3) as outT_pool, \
         tc.tile_pool(name="nrm", bufs=2) as nrm_pool, \
         tc.tile_pool(name="osb", bufs=3) as out_pool:

        from collections import deque

        def make_wo_group(msb, outT_t, mb, db):
            def emit():
                pso = wo_psum.tile([128, 512], F32, tag="wo")
                for cb in range(4):
                    nc.tensor.matmul(
                        pso[:],
                        outT_t[:, cb * 512 + mb * 128: cb * 512 + (mb + 1) * 128],
                        wo_sb[:, cb * D + db * 512: cb * D + (db + 1) * 512],
                        start=(cb == 0), stop=(cb == 3),
                    )
                o_t = out_pool.tile([128, 512], BF16, tag="osb")
                nc.vector.tensor_copy(o_t[:], pso[:])
                nc.sync.dma_start(
                    out[msb * 512 + mb * 128: msb * 512 + (mb + 1) * 128,
                        db * 512:(db + 1) * 512],
                    o_t[:],
                )
            return emit

        def make_normalize(un, r16, outT_t, p):
            # Deferred tail of the normalize: PE outer-product broadcast of
            # the reciprocal row, fused multiply into outT. The reciprocal
            # itself was issued eagerly (it is a ~5us single-lane DVE op that
            # must complete during the next head-pair's j-loop, not block it).
            def emit():
                bps = bps_psum.tile([128, 512], F32, tag="bps")
                nc.tensor.matmul(
                    bps[0:64, :], ones_row[:], r16[:, 0:512],
                    start=True, stop=True, tile_position=(0, 0),
                )
                nc.tensor.matmul(
                    bps[64:128, :], ones_row[:], r16[:, 512:1024],
                    start=True, stop=True, tile_position=(0, 64),
                )
                bc = nrm_pool.tile([64, 1024], BF16, tag="bc")
                nc.vector.tensor_copy(bc[:, 0:512], bps[0:64, :])
                nc.vector.tensor_copy(bc[:, 512:1024], bps[64:128, :])
                nc.vector.tensor_mul(
                    outT_t[0:64, p * 512:(p + 1) * 512], un[0:64, 0:512],
                    bc[:, 0:512],
                )
                nc.vector.tensor_mul(
                    outT_t[64:128, p * 512:(p + 1) * 512], un[0:64, 512:1024],
                    bc[:, 512:1024],
                )
            return emit

        # Deferred-work plumbing: the normalize chain for head-pair p is
        # emitted during p+1's j-loop so its DVE/PE ops never head-block the
        # PE queue, and Wo matmul groups drip in two key-blocks apart so the
        # exp pipeline never drains during an output-projection burst.
        pending_norm = None
        wo_queue = deque()
        js_since_wo = [0]

        def drip_wo():
            js_since_wo[0] += 1
            if wo_queue and js_since_wo[0] >= 2:
                wo_queue.popleft()()
                js_since_wo[0] = 0

        for msb in range(MSB):
            outT_t = outT_pool.tile([128, 2048], BF16, tag="outT")
            for p in range(4):
                acc = acc_psum.tile([128, 1024], F32, tag="acc")
                njb = 4 * msb + 4
                qbase = msb * 2048 + p * 512
                for j in range(njb):
                    if j < 4 * msb:
                        moff, W = 0, 512
                    else:
                        t = j - 4 * msb
                        moff, W = 128 * t, 512 - 128 * t
                    first = (j == 0)
                    last = (j == njb - 1)
                    qlo = qT_sb[0:64, qbase + moff: qbase + moff + W]
                    qhi = qT_sb[64:128, qbase + moff: qbase + moff + W]
                    lt = lt_psum.tile([128, 1024], F32, tag="lt")
                    nc.tensor.matmul(
                        lt[:, 0:W],
                        kT_sb[0:64, j * 128:(j + 1) * 128], qlo,
                        start=True, stop=True, tile_position=(0, 0),
                    )
                    nc.tensor.matmul(
                        lt[:, 512:512 + W],
                        kT_sb[64:128, j * 128:(j + 1) * 128], qhi,
                        start=True, stop=True, tile_position=(64, 0),
                    )
                    pt = pt_pool.tile([128, 1024], BF16, tag="pt")
                    nc.scalar.activation(pt[:], lt[:], Exp, scale=INV_SQRT_DIMK)
                    if j >= 4 * msb:  # diagonal: mask the leading triangle
                        nc.gpsimd.tensor_mul(pt[:, 0:128], pt[:, 0:128],
                                             mask_sb[:])
                        nc.gpsimd.tensor_mul(pt[:, 512:640], pt[:, 512:640],
                                             mask_sb[:])
                    # attn @ v with the denominator folded in (ones col at 64)
                    nc.tensor.matmul(
                        acc[0:65, moff:moff + W],
                        v_sb[:, j * VST: j * VST + 65], pt[:, 0:W],
                        start=first, stop=last,
                    )
                    nc.tensor.matmul(
                        acc[0:65, 512 + moff:512 + moff + W],
                        v_sb[:, j * VST + 65: j * VST + 130], pt[:, 512:512 + W],
                        start=first, stop=last,
                    )
                    drip_wo()
                # First flush the deferred normalize of the previous pair
                # (its reciprocal finished during this j-loop, so nothing
                # here waits on a long DVE chain), THEN queue this pair's
                # acc eviction + slow single-lane reciprocal behind it.
                if pending_norm is not None:
                    pn_msb, pn_p, pn_emit, pn_outT = pending_norm
                    pn_emit()
                    if pn_p == 3:
                        for mb in range(4):
                            for db in range(4):
                                wo_queue.append(
                                    make_wo_group(pn_msb, pn_outT, mb, db))
                un = nrm_pool.tile([128, 1024], BF16, tag="un")
                nc.vector.tensor_copy(un[0:65, :], acc[0:65, :])
                r16 = nrm_pool.tile([1, 1024], BF16, tag="r16")
                with nc.allow_low_precision(reason="softmax denom recip"):
                    nc.vector.reciprocal(r16[:], un[64:65, :])
                pending_norm = (msb, p, make_normalize(un, r16, outT_t, p),
                                outT_t)
        pn_msb, pn_p, pn_emit, pn_outT = pending_norm
        pn_emit()
        for mb in range(4):
            for db in range(4):
                wo_queue.append(make_wo_group(pn_msb, pn_outT, mb, db))
        while wo_queue:
            wo_queue.popleft()()


_NC_CACHE = {}


def get_nc():
    if "nc" not in _NC_CACHE:
        _NC_CACHE["nc"] = build_bass()
    return _NC_CACHE["nc"]


def kernel(inputs_q, inputs_k, inputs_v, Wq, bq, Wk, bk, Wv, bv, Wo, bo):
    inputs_q = np.asarray(inputs_q, np.float32)
    inputs_k = np.asarray(inputs_k, np.float32)
    inputs_v = np.asarray(inputs_v, np.float32)
    Wq = np.asarray(Wq, np.float32)
    Wk = np.asarray(Wk, np.float32)
    Wv = np.asarray(Wv, np.float32)
    Wo = np.asarray(Wo, np.float32)
    bq = np.asarray(bq, np.float32)
    bk = np.asarray(bk, np.float32)
    bv = np.asarray(bv, np.float32)
    bo = np.asarray(bo, np.float32)

    nc = get_nc()
    trimask = np.triu(np.ones((128, 128), np.float32)).astype(NPBF16)
    identity = np.eye(128, dtype=np.float32).astype(NPBF16)

    def blocked(x):
        # [S, D] -> [kc, nsb, 128 (d), 512 (n)] bf16, each chunk contiguous
        return np.ascontiguousarray(
            x.reshape(4, 512, KC, 128).transpose(2, 0, 3, 1).astype(NPBF16)
        )

    xT = {}
    for b in range(B):
        xT[("q", b)] = blocked(inputs_q[b])
        xT[("k", b)] = blocked(inputs_k[b])
        xT[("v", b)] = blocked(inputs_v[b])

    in_maps = []
    for c in range(8):
        b = c // 4
        g0 = 2 * (c % 4)
        g1 = g0 + 1
        # pair-major channel permutation: (head p of g0, head p of g1), p=0..3
        perm = []
        for p in range(HPG):
            perm.extend(range(256 * g0 + 64 * p, 256 * g0 + 64 * p + 64))
            perm.extend(range(256 * g1 + 64 * p, 256 * g1 + 64 * p + 64))
        perm = np.array(perm)
        in_maps.append({
            "xqT": xT[("q", b)],
            "xkT": xT[("k", b)],
            "xvT": xT[("v", b)],
            "wq": Wq[:, perm].astype(NPBF16),
            "wk": Wk[:, 64 * g0: 64 * g0 + 128].astype(NPBF16),
            "wv": Wv[:, 64 * g0: 64 * g0 + 128].astype(NPBF16),
            "wo": Wo[perm, :].astype(NPBF16),
            "bq": np.ascontiguousarray(bq[perm].reshape(CPC, 1)),
            "bk": np.ascontiguousarray(bk[64 * g0: 64 * g0 + 128].reshape(128, 1)),
            "trimask": trimask,
            "ident": identity,
        })

    res = run_bass_kernel_spmd(nc, in_maps, list(range(8)))

    # bv passes through (attention rows sum to 1): out += bv_expand @ Wo + bo
    bv_expand = np.repeat(bv.reshape(NKV, 1, HD), HPG, axis=1).reshape(D)
    corr = (bv_expand.astype(np.float64) @ Wo.astype(np.float64)) + bo

    outp = np.zeros((B, S, D), np.float32)
    for c in range(8):
        outp[c // 4] += res.results[c]["out"].astype(np.float32)
    outp += corr.astype(np.float32)
    return outp


# revision 28
# speedup vs baseline: 1.0904x; 1.0904x over previous
"""GroupedQueryAttention Trainium2 kernel (8-core SPMD, bf16 datapath).

Problem: B=2, S=2048, D=2048, 32 Q heads, 8 KV groups, head_dim=64.
  q = xq @ Wq + bq; k = xk @ Wk + bk; v = xv @ Wv + bv
  logits = q . k / sqrt(512), causal softmax, out = (attn @ v) @ Wo + bo

Sharding: one batch x two KV groups per core (2 batches x 4 group-pairs = 8).
Each core computes its 8 Q heads' attention and a partial output projection
(rows of Wo for its 512 channels); the host sums the 4 partials per batch and
adds the bv/bo corrections (bv passes through softmax linearly since attention
weights sum to 1, so bv_expand @ Wo + bo is exact).

Perf notes vs the fp32 version:
- All matmul operands are bf16 (host casts inputs/weights; PSUM accumulation
  stays fp32): 1 PE cycle/row at any width vs fp32's two-pass LOW_HIGH mode.
- Softmax denominators are folded into the attn@v matmul via a ones column
  appended to each head-group's V block (M=65), killing the dedicated
  ones-vector matmul streams.
- V is projected transposed (weights stationary, x streaming) then flipped
  with PE transpose ops - much cheaper than streaming 128-wide W with x tiles
  as stationary weights.
- The two logit halves of a key block land in one 2-bank PSUM tile so a single
  wide activation does exp for both (fewer Act fixed overheads).
- Reciprocal uses the fast-approx DVE op (f32), downcast to bf16 on the Act
  engine (single-partition DVE ops are lane-serial and slow).
- Input/output DMA spread across scalar/gpsimd/sync queues (~95GB/s each).
- Wo projection of superblock i is emitted after the first head-pair of
  superblock i+1 so its matmuls never head-block the PE queue.
"""

import math
import numpy as np
import ml_dtypes

import concourse.bass as bass
import concourse.mybir as mybir
from concourse import tile
from concourse.bass_utils import run_bass_kernel_spmd
from concourse.vector_clock import ScopedClock

F32 = mybir.dt.float32
BF16 = mybir.dt.bfloat16
NPBF16 = ml_dtypes.bfloat16
B, S, D = 2, 2048, 2048
NKV, HPG, HD = 8, 4, 64
DIMK = 512                 # k/v projection width; also the softmax scale base
CPC = 512                  # q channels per core (2 groups * 4 heads * 64)
KC = D // 128              # 16 k-chunks
MSB = S // 512             # 4 m-superblocks
NB = S // 128              # 16 n-blocks
VST = 130                  # v_sb per-block stride: 64 v_a | 1 | 64 v_b | 1
INV_SQRT_DIMK = 1.0 / math.sqrt(float(DIMK))


# ---------------------------------------------------------------------------
# TileContext tail-drain patch: the bundled neuronxcc walrus rejects
# instructions carrying more than ~2 sync waits ("Too many sync wait
# commands"). Spread the kernel-tail waits over single-wait nops.
def _patched_drain_and_barrier(self, tick_clock, wait_clock):
    nc = self.nc
    collector = nc.sync.nop(nofuse=True)
    wait_clock.add_sem_waits(
        collector.ins, ScopedClock({None: tick_clock.global_clock})
    )
    si = collector.ins.sync_info
    waits = list(si.on_wait) if si is not None and si.on_wait else []
    if waits:
        collector.ins.sync_info = mybir.SyncInfo(
            on_wait=[waits[0]], on_update=list(si.on_update or [])
        )
        for w in waits[1:]:
            extra = nc.sync.nop(nofuse=True)
            extra.ins.sync_info = mybir.SyncInfo(on_wait=[w], on_update=[])
    nc.sync.drain()
    nc.all_engine_barrier()
    assert self.sems is not None
    popped = nc._tile_sem_poison_stack.pop()
    assert popped is self._sem_poison
    nc.clear_and_free_semaphores(list(self.sems.allocated().values()))
    nc.all_engine_barrier()


tile.TileContext._drain_and_barrier = _patched_drain_and_barrier


_MAXW = 1
_NOPID = [0]


def split_excess_waits(nc):
    """Walrus here encodes at most ~1-2 sync waits per instruction; move the
    excess onto preceding same-engine nops (engine order preserves timing)."""
    for f in nc.m.functions:
        for bb in f.blocks:
            out_list = []
            changed = False
            for inst in bb.instructions:
                si = getattr(inst, "sync_info", None)
                waits = list(si.on_wait) if si is not None and si.on_wait else []
                if len(waits) > _MAXW:
                    changed = True
                    for w in waits[:-_MAXW]:
                        _NOPID[0] += 1
                        nop = mybir.InstNoOp(
                            name=f"waitnop-{_NOPID[0]}", ins=[], outs=[],
                            engine=inst.engine,
                        )
                        nop.sync_info = mybir.SyncInfo(on_wait=[w], on_update=[])
                        out_list.append(nop)
                    inst.sync_info = mybir.SyncInfo(
                        on_wait=waits[-_MAXW:], on_update=list(si.on_update or [])
                    )
                out_list.append(inst)
            if changed:
                bb.instructions[:] = out_list
# ---------------------------------------------------------------------------


def build_bass():
    nc = bass.Bass()
    # x inputs arrive pre-blocked [kc, nsb, 128, 512] so every chunk DMA is
    # one fully linear 128KB read (strided 1KB bf16 lines run at half rate).
    xqT = nc.dram_tensor("xqT", [KC, 4, 128, 512], BF16, kind="ExternalInput")
    xkT = nc.dram_tensor("xkT", [KC, 4, 128, 512], BF16, kind="ExternalInput")
    xvT = nc.dram_tensor("xvT", [KC, 4, 128, 512], BF16, kind="ExternalInput")
    wq = nc.dram_tensor("wq", [D, CPC], BF16, kind="ExternalInput")
    wk = nc.dram_tensor("wk", [D, 128], BF16, kind="ExternalInput")
    wv = nc.dram_tensor("wv", [D, 128], BF16, kind="ExternalInput")
    wo = nc.dram_tensor("wo", [CPC, D], BF16, kind="ExternalInput")
    bq = nc.dram_tensor("bq", [CPC, 1], F32, kind="ExternalInput")
    bk = nc.dram_tensor("bk", [128, 1], F32, kind="ExternalInput")
    trimask = nc.dram_tensor("trimask", [128, 128], BF16, kind="ExternalInput")
    ident = nc.dram_tensor("ident", [128, 128], BF16, kind="ExternalInput")
    out = nc.dram_tensor("out", [S, D], BF16, kind="ExternalOutput")

    from contextlib import ExitStack
    with tile.TileContext(nc) as tc, ExitStack() as ctx:
        build_body(ctx, tc, xqT, xkT, xvT, wq, wk, wv, wo, bq, bk,
                   trimask, ident, out)
    split_excess_waits(nc)
    return nc


def build_body(ctx, tc, xqT, xkT, xvT, wq, wk, wv, wo, bq, bk,
               trimask, ident, out):
    nc = tc.nc
    Exp = mybir.ActivationFunctionType.Exp
    Ident = mybir.ActivationFunctionType.Identity
    Copy = mybir.ActivationFunctionType.Copy

    const = ctx.enter_context(tc.tile_pool(name="const", bufs=1))
    wq_sb = const.tile([128, KC * CPC], BF16, tag="wq")      # [128, 8192]
    wk_sb = const.tile([128, KC * 128], BF16, tag="wk")      # [128, 2048]
    wv_sb = const.tile([128, KC * 128], BF16, tag="wv")      # [128, 2048]
    wo_sb = const.tile([128, 4 * D], BF16, tag="wo")         # [128, 8192]
    kT_sb = const.tile([128, S], BF16, tag="kT")             # [128, 2048]
    v_sb = const.tile([128, NB * VST], BF16, tag="v")        # [128, 2080]
    qT_sb = const.tile([128, 4 * S], BF16, tag="qT")         # [128, 8192]
    bq_sb = const.tile([128, 4], F32, tag="bq")
    bk_sb = const.tile([128, 1], F32, tag="bk")
    mask_sb = const.tile([128, 128], BF16, tag="mask")
    ident_sb = const.tile([128, 128], BF16, tag="ident")
    ones_row = const.tile([1, 64], BF16, tag="ones_row")

    # Weight / bias / mask loads: each weight rides its consumer stream's
    # queue just ahead of the x chunks (xk->gpsimd, xv->sync, xq->scalar);
    # wo is loaded late (emitted after the projection loop) on sync.
    nc.gpsimd.dma_start(
        wk_sb[:].rearrange("p (kc c) -> p kc c", kc=KC),
        wk.rearrange("(kc p) c -> p kc c", p=128),
    )
    nc.sync.dma_start(
        wv_sb[:].rearrange("p (kc c) -> p kc c", kc=KC),
        wv.rearrange("(kc p) c -> p kc c", p=128),
    )
    nc.scalar.dma_start(
        wq_sb[:].rearrange("p (kc c) -> p kc c", kc=KC),
        wq.rearrange("(kc p) c -> p kc c", p=128),
    )
    nc.scalar.dma_start(
        bq_sb[:].rearrange("p (cb o) -> p cb o", cb=4),
        bq.rearrange("(cb p) o -> p cb o", p=128),
    )
    nc.scalar.dma_start(bk_sb[:], bk[:])
    nc.scalar.dma_start(mask_sb[:], trimask[:])
    nc.scalar.dma_start(ident_sb[:], ident[:])
    nc.vector.memset(v_sb[:], 1.0)   # ones columns at 64/129 of each block
    nc.vector.memset(ones_row[:], 1.0)

    # ---------------- Phase 1-3: projections ----------------
    with tc.tile_pool(name="proj_psum", bufs=6, space="PSUM") as proj_psum, \
         tc.tile_pool(name="tp_psum", bufs=2, space="PSUM") as tp_psum, \
         tc.tile_pool(name="xin", bufs=5) as xin_pool, \
         tc.tile_pool(name="xvin", bufs=5) as xvin_pool, \
         tc.tile_pool(name="vt", bufs=2) as vt_pool:

        # K/V/Q projections interleaved per 512-token superblock so that all
        # three x input streams (gpsimd/sync/scalar queues) run concurrently.
        for nsb in range(4):
            # K: kT[c=128, n] accumulated over k-chunks, bias bk.
            ps = proj_psum.tile([128, 512], F32, tag="ps")
            for kc in range(KC):
                xk_t = xin_pool.tile([128, 512], BF16, tag="xk")
                nc.gpsimd.dma_start(
                    xk_t[:], xkT[kc, nsb]
                )
                nc.tensor.matmul(
                    ps[:], wk_sb[:, kc * 128:(kc + 1) * 128], xk_t[:],
                    start=(kc == 0), stop=(kc == KC - 1),
                )
            nc.scalar.activation(
                kT_sb[:, nsb * 512:(nsb + 1) * 512], ps[:], Ident, bias=bk_sb[:]
            )

            # V: projected transposed (vT[c, n]) with wv stationary, then PE
            # transposes into v natural layout with interleaved ones columns.
            ps = proj_psum.tile([128, 512], F32, tag="ps")
            for kc in range(KC):
                xv_t = xvin_pool.tile([128, 512], BF16, tag="xv")
                nc.sync.dma_start(
                    xv_t[:], xvT[kc, nsb]
                )
                nc.tensor.matmul(
                    ps[:], wv_sb[:, kc * 128:(kc + 1) * 128], xv_t[:],
                    start=(kc == 0), stop=(kc == KC - 1),
                )
            vT_t = vt_pool.tile([128, 512], BF16, tag="vt")
            nc.vector.tensor_copy(vT_t[:], ps[:])
            for t in range(4):
                j = nsb * 4 + t
                tp = tp_psum.tile([128, 128], BF16, tag="tp")
                nc.tensor.transpose(tp[:], vT_t[:, t * 128:(t + 1) * 128],
                                    ident_sb[:])
                nc.vector.tensor_copy(v_sb[:, j * VST: j * VST + 64],
                                      tp[:, 0:64])
                nc.vector.tensor_copy(v_sb[:, j * VST + 65: j * VST + 129],
                                      tp[:, 64:128])

            # Q: qT[c, m], layout [msb][cb][512], bias bq (msb == nsb).
            msb = nsb
            pss = [proj_psum.tile([128, 512], F32, tag="ps", name=f"psq{cb}")
                   for cb in range(4)]
            for kc in range(KC):
                xq_t = xin_pool.tile([128, 512], BF16, tag="xq")
                nc.scalar.dma_start(
                    xq_t[:], xqT[kc, msb]
                )
                for cb in range(4):
                    nc.tensor.matmul(
                        pss[cb][:],
                        wq_sb[:, kc * CPC + cb * 128: kc * CPC + (cb + 1) * 128],
                        xq_t[:],
                        start=(kc == 0), stop=(kc == KC - 1),
                    )
            for cb in range(4):
                nc.scalar.activation(
                    qT_sb[:, msb * 2048 + cb * 512: msb * 2048 + (cb + 1) * 512],
                    pss[cb][:], Ident, bias=bq_sb[:, cb:cb + 1],
                )

    # wo lands during early attention; first consumer is ~2/3 in.
    nc.sync.dma_start(
        wo_sb[:].rearrange("p (cb d) -> p cb d", cb=4),
        wo.rearrange("(cb p) d -> p cb d", p=128),
    )

    # ---------------- Phase 4: attention + output projection ----------------
    with tc.tile_pool(name="lt_psum", bufs=2, space="PSUM") as lt_psum, \
         tc.tile_pool(name="acc_psum", bufs=1, space="PSUM") as acc_psum, \
         tc.tile_pool(name="aux_psum", bufs=2, space="PSUM") as aux_psum, \
         tc.tile_pool(name="pt", bufs=3) as pt_pool, \
         tc.tile_pool(name="outT", bufs=3) as outT_pool, \
         tc.tile_pool(name="nrm", bufs=2) as nrm_pool, \
         tc.tile_pool(name="osb", bufs=3) as out_pool:

        from collections import deque

        def make_wo_group(msb, outT_t, mb, db):
            def emit():
                pso = aux_psum.tile([128, 512], F32, tag="aux")
                for cb in range(4):
                    nc.tensor.matmul(
                        pso[:],
                        outT_t[:, cb * 512 + mb * 128: cb * 512 + (mb + 1) * 128],
                        wo_sb[:, cb * D + db * 512: cb * D + (db + 1) * 512],
                        start=(cb == 0), stop=(cb == 3),
                    )
                o_t = out_pool.tile([128, 512], BF16, tag="osb")
                nc.vector.tensor_copy(o_t[:], pso[:])
                nc.sync.dma_start(
                    out[msb * 512 + mb * 128: msb * 512 + (mb + 1) * 128,
                        db * 512:(db + 1) * 512],
                    o_t[:],
                )
            return emit

        def make_normalize(un, r16, outT_t, p):
            # Deferred tail of the normalize: PE outer-product broadcast of
            # the reciprocal row, fused multiply into outT. The reciprocal
            # itself was issued eagerly (it is a ~5us single-lane DVE op that
            # must complete during the next head-pair's j-loop, not block it).
            def emit():
                bps = aux_psum.tile([128, 512], F32, tag="aux")
                nc.tensor.matmul(
                    bps[0:64, :], ones_row[:], r16[:, 0:512],
                    start=True, stop=True, tile_position=(0, 0),
                )
                nc.tensor.matmul(
                    bps[64:128, :], ones_row[:], r16[:, 512:1024],
                    start=True, stop=True, tile_position=(0, 64),
                )
                bc = nrm_pool.tile([64, 1024], BF16, tag="bc")
                nc.vector.tensor_copy(bc[:, 0:512], bps[0:64, :])
                nc.vector.tensor_copy(bc[:, 512:1024], bps[64:128, :])
                nc.vector.tensor_mul(
                    outT_t[0:64, p * 512:(p + 1) * 512], un[0:64, 0:512],
                    bc[:, 0:512],
                )
                nc.vector.tensor_mul(
                    outT_t[64:128, p * 512:(p + 1) * 512], un[0:64, 512:1024],
                    bc[:, 512:1024],
                )
            return emit

        # Deferred-work plumbing: the normalize chain for head-pair p is
        # emitted during p+1's j-loop so its DVE/PE ops never head-block the
        # PE queue, and Wo matmul groups drip in two key-blocks apart so the
        # exp pipeline never drains during an output-projection burst.
        pending_norm = None
        wo_queue = deque()
        js_since_wo = [0]

        def drip_wo():
            js_since_wo[0] += 1
            if wo_queue and js_since_wo[0] >= 2:
                wo_queue.popleft()()
                js_since_wo[0] = 0

        for msb in range(MSB):
            outT_t = outT_pool.tile([128, 2048], BF16, tag="outT")
            for p in range(4):
                acc = acc_psum.tile([128, 1024], F32, tag="acc")
                njb = 4 * msb + 4
                qbase = msb * 2048 + p * 512
                for j in range(njb):
                    if j < 4 * msb:
                        moff, W = 0, 512
                    else:
                        t = j - 4 * msb
                        moff, W = 128 * t, 512 - 128 * t
                    first = (j == 0)
                    last = (j == njb - 1)
                    qlo = qT_sb[0:64, qbase + moff: qbase + moff + W]
                    qhi = qT_sb[64:128, qbase + moff: qbase + moff + W]
                    lt = lt_psum.tile([128, 1024], F32, tag="lt")
                    nc.tensor.matmul(
                        lt[:, 0:W],
                        kT_sb[0:64, j * 128:(j + 1) * 128], qlo,
                        start=True, stop=True, tile_position=(0, 0),
                    )
                    nc.tensor.matmul(
                        lt[:, 512:512 + W],
                        kT_sb[64:128, j * 128:(j + 1) * 128], qhi,
                        start=True, stop=True, tile_position=(64, 0),
                    )
                    pt = pt_pool.tile([128, 1024], BF16, tag="pt")
                    nc.scalar.activation(pt[:], lt[:], Exp, scale=INV_SQRT_DIMK)
                    if j >= 4 * msb:  # diagonal: mask the leading triangle
                        nc.gpsimd.tensor_mul(pt[:, 0:128], pt[:, 0:128],
                                             mask_sb[:])
                        nc.gpsimd.tensor_mul(pt[:, 512:640], pt[:, 512:640],
                                             mask_sb[:])
                    # attn @ v with the denominator folded in (ones col at 64)
                    nc.tensor.matmul(
                        acc[0:65, moff:moff + W],
                        v_sb[:, j * VST: j * VST + 65], pt[:, 0:W],
                        start=first, stop=last,
                    )
                    nc.tensor.matmul(
                        acc[0:65, 512 + moff:512 + moff + W],
                        v_sb[:, j * VST + 65: j * VST + 130], pt[:, 512:512 + W],
                        start=first, stop=last,
                    )
                    drip_wo()
                # DVE order at a pair boundary: (1) evict acc so its single
                # PSUM buffer frees ASAP, (2) flush the deferred normalize of
                # the previous pair (its reciprocal finished during this
                # j-loop), (3) start this pair's slow single-lane reciprocal
                # last so it hides under the next j-loop.
                un = nrm_pool.tile([128, 1024], BF16, tag="un")
                nc.vector.tensor_copy(un[0:65, :], acc[0:65, :])
                if pending_norm is not None:
                    pn_msb, pn_p, pn_emit, pn_outT = pending_norm
                    pn_emit()
                    if pn_p == 3:
                        for mb in range(4):
                            for db in range(4):
                                wo_queue.append(
                                    make_wo_group(pn_msb, pn_outT, mb, db))
                r16 = nrm_pool.tile([1, 1024], BF16, tag="r16")
                with nc.allow_low_precision(reason="softmax denom recip"):
                    nc.vector.reciprocal(r16[:], un[64:65, :])
                pending_norm = (msb, p, make_normalize(un, r16, outT_t, p),
                                outT_t)
        pn_msb, pn_p, pn_emit, pn_outT = pending_norm
        pn_emit()
        for mb in range(4):
            for db in range(4):
                wo_queue.append(make_wo_group(pn_msb, pn_outT, mb, db))
        while wo_queue:
            wo_queue.popleft()()


_NC_CACHE = {}


def get_nc():
    if "nc" not in _NC_CACHE:
        _NC_CACHE["nc"] = build_bass()
    return _NC_CACHE["nc"]


def kernel(inputs_q, inputs_k, inputs_v, Wq, bq, Wk, bk, Wv, bv, Wo, bo):
    inputs_q = np.asarray(inputs_q, np.float32)
    inputs_k = np.asarray(inputs_k, np.float32)
    inputs_v = np.asarray(inputs_v, np.float32)
    Wq = np.asarray(Wq, np.float32)
    Wk = np.asarray(Wk, np.float32)
    Wv = np.asarray(Wv, np.float32)
    Wo = np.asarray(Wo, np.float32)
    bq = np.asarray(bq, np.float32)
    bk = np.asarray(bk, np.float32)
    bv = np.asarray(bv, np.float32)
    bo = np.asarray(bo, np.float32)

    nc = get_nc()
    trimask = np.triu(np.ones((128, 128), np.float32)).astype(NPBF16)
    identity = np.eye(128, dtype=np.float32).astype(NPBF16)

    def blocked(x):
        # [S, D] -> [kc, nsb, 128 (d), 512 (n)] bf16, each chunk contiguous
        return np.ascontiguousarray(
            x.reshape(4, 512, KC, 128).transpose(2, 0, 3, 1).astype(NPBF16)
        )

    xT = {}
    for b in range(B):
        xT[("q", b)] = blocked(inputs_q[b])
        xT[("k", b)] = blocked(inputs_k[b])
        xT[("v", b)] = blocked(inputs_v[b])

    in_maps = []
    for c in range(8):
        b = c // 4
        g0 = 2 * (c % 4)
        g1 = g0 + 1
        # pair-major channel permutation: (head p of g0, head p of g1), p=0..3
        perm = []
        for p in range(HPG):
            perm.extend(range(256 * g0 + 64 * p, 256 * g0 + 64 * p + 64))
            perm.extend(range(256 * g1 + 64 * p, 256 * g1 + 64 * p + 64))
        perm = np.array(perm)
        in_maps.append({
            "xqT": xT[("q", b)],
            "xkT": xT[("k", b)],
            "xvT": xT[("v", b)],
            "wq": Wq[:, perm].astype(NPBF16),
            "wk": Wk[:, 64 * g0: 64 * g0 + 128].astype(NPBF16),
            "wv": Wv[:, 64 * g0: 64 * g0 + 128].astype(NPBF16),
            "wo": Wo[perm, :].astype(NPBF16),
            "bq": np.ascontiguousarray(bq[perm].reshape(CPC, 1)),
            "bk": np.ascontiguousarray(bk[64 * g0: 64 * g0 + 128].reshape(128, 1)),
            "trimask": trimask,
            "ident": identity,
        })

    res = run_bass_kernel_spmd(nc, in_maps, list(range(8)))

    # bv passes through (attention rows sum to 1): out += bv_expand @ Wo + bo
    bv_expand = np.repeat(bv.reshape(NKV, 1, HD), HPG, axis=1).reshape(D)
    corr = (bv_expand.astype(np.float64) @ Wo.astype(np.float64)) + bo

    outp = np.zeros((B, S, D), np.float32)
    for c in range(8):
        outp[c // 4] += res.results[c]["out"].astype(np.float32)
    outp += corr.astype(np.float32)
    return outp


# revision 30
# speedup vs baseline: 1.1013x; 1.0100x over previous
"""GroupedQueryAttention Trainium2 kernel (8-core SPMD, bf16 datapath).

Problem: B=2, S=2048, D=2048, 32 Q heads, 8 KV groups, head_dim=64.
  q = xq @ Wq + bq; k = xk @ Wk + bk; v = xv @ Wv + bv
  logits = q . k / sqrt(512), causal softmax, out = (attn @ v) @ Wo + bo

Sharding: one batch x two KV groups per core (2 batches x 4 group-pairs = 8).
Each core computes its 8 Q heads' attention and a partial output projection
(rows of Wo for its 512 channels); the host sums the 4 partials per batch and
adds the bv/bo corrections (bv passes through softmax linearly since attention
weights sum to 1, so bv_expand @ Wo + bo is exact).

Perf notes vs the fp32 version:
- All matmul operands are bf16 (host casts inputs/weights; PSUM accumulation
  stays fp32): 1 PE cycle/row at any width vs fp32's two-pass LOW_HIGH mode.
- Softmax denominators are folded into the attn@v matmul via a ones column
  appended to each head-group's V block (M=65), killing the dedicated
  ones-vector matmul streams.
- V is projected transposed (weights stationary, x streaming) then flipped
  with PE transpose ops - much cheaper than streaming 128-wide W with x tiles
  as stationary weights.
- The two logit halves of a key block land in one 2-bank PSUM tile so a single
  wide activation does exp for both (fewer Act fixed overheads).
- Reciprocal uses the fast-approx DVE op (f32), downcast to bf16 on the Act
  engine (single-partition DVE ops are lane-serial and slow).
- Input/output DMA spread across scalar/gpsimd/sync queues (~95GB/s each).
- Wo projection of superblock i is emitted after the first head-pair of
  superblock i+1 so its matmuls never head-block the PE queue.
"""

import math
import numpy as np
import ml_dtypes

import concourse.bass as bass
import concourse.mybir as mybir
from concourse import tile
from concourse.bass_utils import run_bass_kernel_spmd
from concourse.vector_clock import ScopedClock

F32 = mybir.dt.float32
BF16 = mybir.dt.bfloat16
NPBF16 = ml_dtypes.bfloat16
B, S, D = 2, 2048, 2048
NKV, HPG, HD = 8, 4, 64
DIMK = 512                 # k/v projection width; also the softmax scale base
CPC = 512                  # q channels per core (2 groups * 4 heads * 64)
KC = D // 128              # 16 k-chunks
MSB = S // 512             # 4 m-superblocks
NB = S // 128              # 16 n-blocks
VST = 130                  # v_sb per-block stride: 64 v_a | 1 | 64 v_b | 1
INV_SQRT_DIMK = 1.0 / math.sqrt(float(DIMK))


# ---------------------------------------------------------------------------
# TileContext tail-drain patch: the bundled neuronxcc walrus rejects
# instructions carrying more than ~2 sync waits ("Too many sync wait
# commands"). Spread the kernel-tail waits over single-wait nops.
def _patched_drain_and_barrier(self, tick_clock, wait_clock):
    nc = self.nc
    collector = nc.sync.nop(nofuse=True)
    wait_clock.add_sem_waits(
        collector.ins, ScopedClock({None: tick_clock.global_clock})
    )
    si = collector.ins.sync_info
    waits = list(si.on_wait) if si is not None and si.on_wait else []
    if waits:
        collector.ins.sync_info = mybir.SyncInfo(
            on_wait=[waits[0]], on_update=list(si.on_update or [])
        )
        for w in waits[1:]:
            extra = nc.sync.nop(nofuse=True)
            extra.ins.sync_info = mybir.SyncInfo(on_wait=[w], on_update=[])
    nc.sync.drain()
    nc.all_engine_barrier()
    assert self.sems is not None
    popped = nc._tile_sem_poison_stack.pop()
    assert popped is self._sem_poison
    nc.clear_and_free_semaphores(list(self.sems.allocated().values()))
    nc.all_engine_barrier()


tile.TileContext._drain_and_barrier = _patched_drain_and_barrier


_MAXW = 1
_NOPID = [0]


def split_excess_waits(nc):
    """Walrus here encodes at most ~1-2 sync waits per instruction; move the
    excess onto preceding same-engine nops (engine order preserves timing)."""
    for f in nc.m.functions:
        for bb in f.blocks:
            out_list = []
            changed = False
            for inst in bb.instructions:
                si = getattr(inst, "sync_info", None)
                waits = list(si.on_wait) if si is not None and si.on_wait else []
                if len(waits) > _MAXW:
                    changed = True
                    for w in waits[:-_MAXW]:
                        _NOPID[0] += 1
                        nop = mybir.InstNoOp(
                            name=f"waitnop-{_NOPID[0]}", ins=[], outs=[],
                            engine=inst.engine,
                        )
                        nop.sync_info = mybir.SyncInfo(on_wait=[w], on_update=[])
                        out_list.append(nop)
                    inst.sync_info = mybir.SyncInfo(
                        on_wait=waits[-_MAXW:], on_update=list(si.on_update or [])
                    )
                out_list.append(inst)
            if changed:
                bb.instructions[:] = out_list
# ---------------------------------------------------------------------------


def build_bass():
    nc = bass.Bass()
    # x inputs arrive pre-blocked [kc, nsb, 128, 512] so every chunk DMA is
    # one fully linear 128KB read (strided 1KB bf16 lines run at half rate).
    xqT = nc.dram_tensor("xqT", [KC, 4, 128, 512], BF16, kind="ExternalInput")
    xkT = nc.dram_tensor("xkT", [KC, 4, 128, 512], BF16, kind="ExternalInput")
    xvT = nc.dram_tensor("xvT", [KC, 4, 128, 512], BF16, kind="ExternalInput")
    wq = nc.dram_tensor("wq", [D, CPC], BF16, kind="ExternalInput")
    wk = nc.dram_tensor("wk", [D, 128], BF16, kind="ExternalInput")
    wv = nc.dram_tensor("wv", [D, 128], BF16, kind="ExternalInput")
    wo = nc.dram_tensor("wo", [CPC, D], BF16, kind="ExternalInput")
    bq = nc.dram_tensor("bq", [CPC, 1], F32, kind="ExternalInput")
    bk = nc.dram_tensor("bk", [128, 1], F32, kind="ExternalInput")
    trimask = nc.dram_tensor("trimask", [128, 128], BF16, kind="ExternalInput")
    ident = nc.dram_tensor("ident", [128, 128], BF16, kind="ExternalInput")
    out = nc.dram_tensor("out", [S, D], BF16, kind="ExternalOutput")

    from contextlib import ExitStack
    with tile.TileContext(nc) as tc, ExitStack() as ctx:
        build_body(ctx, tc, xqT, xkT, xvT, wq, wk, wv, wo, bq, bk,
                   trimask, ident, out)
    split_excess_waits(nc)
    return nc


def build_body(ctx, tc, xqT, xkT, xvT, wq, wk, wv, wo, bq, bk,
               trimask, ident, out):
    nc = tc.nc
    Exp = mybir.ActivationFunctionType.Exp
    Ident = mybir.ActivationFunctionType.Identity
    Copy = mybir.ActivationFunctionType.Copy

    const = ctx.enter_context(tc.tile_pool(name="const", bufs=1))
    wq_sb = const.tile([128, KC * CPC], BF16, tag="wq")      # [128, 8192]
    wk_sb = const.tile([128, KC * 128], BF16, tag="wk")      # [128, 2048]
    wv_sb = const.tile([128, KC * 128], BF16, tag="wv")      # [128, 2048]
    wo_sb = const.tile([128, 4 * D], BF16, tag="wo")         # [128, 8192]
    kT_sb = const.tile([128, S], BF16, tag="kT")             # [128, 2048]
    v_sb = const.tile([128, NB * VST], BF16, tag="v")        # [128, 2080]
    qT_sb = const.tile([128, 4 * S], BF16, tag="qT")         # [128, 8192]
    bq_sb = const.tile([128, 4], F32, tag="bq")
    bk_sb = const.tile([128, 1], F32, tag="bk")
    mask_sb = const.tile([128, 128], BF16, tag="mask")
    ident_sb = const.tile([128, 128], BF16, tag="ident")
    ones_row = const.tile([1, 64], BF16, tag="ones_row")

    # Weight / bias / mask loads: each weight rides its consumer stream's
    # queue just ahead of the x chunks (xk->gpsimd, xv->sync, xq->scalar);
    # wo is loaded late (emitted after the projection loop) on sync.
    nc.gpsimd.dma_start(
        wk_sb[:].rearrange("p (kc c) -> p kc c", kc=KC),
        wk.rearrange("(kc p) c -> p kc c", p=128),
    )
    nc.sync.dma_start(
        wv_sb[:].rearrange("p (kc c) -> p kc c", kc=KC),
        wv.rearrange("(kc p) c -> p kc c", p=128),
    )
    nc.scalar.dma_start(
        wq_sb[:].rearrange("p (kc c) -> p kc c", kc=KC),
        wq.rearrange("(kc p) c -> p kc c", p=128),
    )
    nc.scalar.dma_start(
        bq_sb[:].rearrange("p (cb o) -> p cb o", cb=4),
        bq.rearrange("(cb p) o -> p cb o", p=128),
    )
    nc.scalar.dma_start(bk_sb[:], bk[:])
    nc.scalar.dma_start(mask_sb[:], trimask[:])
    nc.scalar.dma_start(ident_sb[:], ident[:])
    nc.vector.memset(v_sb[:], 1.0)   # ones columns at 64/129 of each block
    nc.vector.memset(ones_row[:], 1.0)

    # ---------------- Phase 1-3: projections ----------------
    with tc.tile_pool(name="proj_psum", bufs=6, space="PSUM") as proj_psum, \
         tc.tile_pool(name="tp_psum", bufs=2, space="PSUM") as tp_psum, \
         tc.tile_pool(name="xin", bufs=5) as xin_pool, \
         tc.tile_pool(name="xvin", bufs=5) as xvin_pool, \
         tc.tile_pool(name="vt", bufs=2) as vt_pool:

        # K/V/Q projections interleaved per 512-token superblock so that all
        # three x input streams (gpsimd/sync/scalar queues) run concurrently.
        for nsb in range(4):
            # K: kT[c=128, n] accumulated over k-chunks, bias bk.
            ps = proj_psum.tile([128, 512], F32, tag="ps")
            for kc in range(KC):
                xk_t = xin_pool.tile([128, 512], BF16, tag="xk")
                nc.gpsimd.dma_start(
                    xk_t[:], xkT[kc, nsb]
                )
                nc.tensor.matmul(
                    ps[:], wk_sb[:, kc * 128:(kc + 1) * 128], xk_t[:],
                    start=(kc == 0), stop=(kc == KC - 1),
                )
            nc.scalar.activation(
                kT_sb[:, nsb * 512:(nsb + 1) * 512], ps[:], Ident, bias=bk_sb[:]
            )

            # V: projected transposed (vT[c, n]) with wv stationary, then PE
            # transposes into v natural layout with interleaved ones columns.
            ps = proj_psum.tile([128, 512], F32, tag="ps")
            for kc in range(KC):
                xv_t = xvin_pool.tile([128, 512], BF16, tag="xv")
                nc.sync.dma_start(
                    xv_t[:], xvT[kc, nsb]
                )
                nc.tensor.matmul(
                    ps[:], wv_sb[:, kc * 128:(kc + 1) * 128], xv_t[:],
                    start=(kc == 0), stop=(kc == KC - 1),
                )
            vT_t = vt_pool.tile([128, 512], BF16, tag="vt")
            nc.vector.tensor_copy(vT_t[:], ps[:])
            for t in range(4):
                j = nsb * 4 + t
                tp = tp_psum.tile([128, 128], BF16, tag="tp")
                nc.tensor.transpose(tp[:], vT_t[:, t * 128:(t + 1) * 128],
                                    ident_sb[:])
                nc.vector.tensor_copy(v_sb[:, j * VST: j * VST + 64],
                                      tp[:, 0:64])
                nc.vector.tensor_copy(v_sb[:, j * VST + 65: j * VST + 129],
                                      tp[:, 64:128])

            # Q: qT[c, m], layout [msb][cb][512], bias bq (msb == nsb).
            msb = nsb
            pss = [proj_psum.tile([128, 512], F32, tag="ps", name=f"psq{cb}")
                   for cb in range(4)]
            for kc in range(KC):
                xq_t = xin_pool.tile([128, 512], BF16, tag="xq")
                nc.scalar.dma_start(
                    xq_t[:], xqT[kc, msb]
                )
                for cb in range(4):
                    nc.tensor.matmul(
                        pss[cb][:],
                        wq_sb[:, kc * CPC + cb * 128: kc * CPC + (cb + 1) * 128],
                        xq_t[:],
                        start=(kc == 0), stop=(kc == KC - 1),
                    )
            for cb in range(4):
                nc.scalar.activation(
                    qT_sb[:, msb * 2048 + cb * 512: msb * 2048 + (cb + 1) * 512],
                    pss[cb][:], Ident, bias=bq_sb[:, cb:cb + 1],
                )

    # wo lands during early attention; first consumer is ~2/3 in.
    nc.sync.dma_start(
        wo_sb[:].rearrange("p (cb d) -> p cb d", cb=4),
        wo.rearrange("(cb p) d -> p cb d", p=128),
    )

    # ---------------- Phase 4: attention + output projection ----------------
    with tc.tile_pool(name="lt_psum", bufs=2, space="PSUM") as lt_psum, \
         tc.tile_pool(name="acc_psum", bufs=1, space="PSUM") as acc_psum, \
         tc.tile_pool(name="aux_psum", bufs=2, space="PSUM") as aux_psum, \
         tc.tile_pool(name="pt", bufs=3) as pt_pool, \
         tc.tile_pool(name="outT", bufs=3) as outT_pool, \
         tc.tile_pool(name="nrm", bufs=3) as nrm_pool, \
         tc.tile_pool(name="osb", bufs=3) as out_pool:

        from collections import deque

        def make_wo_group(msb, outT_t, mb, db):
            def emit():
                pso = aux_psum.tile([128, 512], F32, tag="aux")
                for cb in range(4):
                    nc.tensor.matmul(
                        pso[:],
                        outT_t[:, cb * 512 + mb * 128: cb * 512 + (mb + 1) * 128],
                        wo_sb[:, cb * D + db * 512: cb * D + (db + 1) * 512],
                        start=(cb == 0), stop=(cb == 3),
                    )
                o_t = out_pool.tile([128, 512], BF16, tag="osb")
                nc.vector.tensor_copy(o_t[:], pso[:])
                nc.sync.dma_start(
                    out[msb * 512 + mb * 128: msb * 512 + (mb + 1) * 128,
                        db * 512:(db + 1) * 512],
                    o_t[:],
                )
            return emit

        def make_normalize(un, r16, outT_t, p):
            # Deferred tail of the normalize: PE outer-product broadcast of
            # the reciprocal row, fused multiply into outT. The reciprocal
            # itself was issued eagerly (it is a ~5us single-lane DVE op that
            # must complete during the next head-pair's j-loop, not block it).
            def emit():
                bps = aux_psum.tile([128, 512], F32, tag="aux")
                nc.tensor.matmul(
                    bps[0:64, :], ones_row[:], r16[:, 0:512],
                    start=True, stop=True, tile_position=(0, 0),
                )
                nc.tensor.matmul(
                    bps[64:128, :], ones_row[:], r16[:, 512:1024],
                    start=True, stop=True, tile_position=(0, 64),
                )
                bc = nrm_pool.tile([64, 1024], BF16, tag="bc")
                nc.vector.tensor_copy(bc[:, 0:512], bps[0:64, :])
                nc.vector.tensor_copy(bc[:, 512:1024], bps[64:128, :])
                nc.vector.tensor_mul(
                    outT_t[0:64, p * 512:(p + 1) * 512], un[0:64, 0:512],
                    bc[:, 0:512],
                )
                nc.vector.tensor_mul(
                    outT_t[64:128, p * 512:(p + 1) * 512], un[0:64, 512:1024],
                    bc[:, 512:1024],
                )
            return emit

        # Deferred-work plumbing: the normalize chain for head-pair p is
        # emitted during p+1's j-loop so its DVE/PE ops never head-block the
        # PE queue, and Wo matmul groups drip in two key-blocks apart so the
        # exp pipeline never drains during an output-projection burst.
        pending_norm = deque()
        wo_queue = deque()
        js_since_wo = [0]

        def drip_wo():
            js_since_wo[0] += 1
            if wo_queue and js_since_wo[0] >= 2:
                wo_queue.popleft()()
                js_since_wo[0] = 0

        for msb in range(MSB):
            outT_t = outT_pool.tile([128, 2048], BF16, tag="outT")
            for p in range(4):
                acc = acc_psum.tile([128, 1024], F32, tag="acc")
                njb = 4 * msb + 4
                qbase = msb * 2048 + p * 512
                for j in range(njb):
                    if j < 4 * msb:
                        moff, W = 0, 512
                    else:
                        t = j - 4 * msb
                        moff, W = 128 * t, 512 - 128 * t
                    first = (j == 0)
                    last = (j == njb - 1)
                    qlo = qT_sb[0:64, qbase + moff: qbase + moff + W]
                    qhi = qT_sb[64:128, qbase + moff: qbase + moff + W]
                    lt = lt_psum.tile([128, 1024], F32, tag="lt")
                    nc.tensor.matmul(
                        lt[:, 0:W],
                        kT_sb[0:64, j * 128:(j + 1) * 128], qlo,
                        start=True, stop=True, tile_position=(0, 0),
                    )
                    nc.tensor.matmul(
                        lt[:, 512:512 + W],
                        kT_sb[64:128, j * 128:(j + 1) * 128], qhi,
                        start=True, stop=True, tile_position=(64, 0),
                    )
                    pt = pt_pool.tile([128, 1024], BF16, tag="pt")
                    nc.scalar.activation(pt[:], lt[:], Exp, scale=INV_SQRT_DIMK)
                    if j >= 4 * msb:  # diagonal: mask the leading triangle
                        nc.gpsimd.tensor_mul(pt[:, 0:128], pt[:, 0:128],
                                             mask_sb[:])
                        nc.gpsimd.tensor_mul(pt[:, 512:640], pt[:, 512:640],
                                             mask_sb[:])
                    # attn @ v with the denominator folded in (ones col at 64)
                    nc.tensor.matmul(
                        acc[0:65, moff:moff + W],
                        v_sb[:, j * VST: j * VST + 65], pt[:, 0:W],
                        start=first, stop=last,
                    )
                    nc.tensor.matmul(
                        acc[0:65, 512 + moff:512 + moff + W],
                        v_sb[:, j * VST + 65: j * VST + 130], pt[:, 512:512 + W],
                        start=first, stop=last,
                    )
                    drip_wo()
                # DVE order at a pair boundary: (1) evict acc so its single
                # PSUM buffer frees ASAP, (2) flush the deferred normalize of
                # the previous pair (its reciprocal finished during this
                # j-loop), (3) start this pair's slow single-lane reciprocal
                # last so it hides under the next j-loop.
                un = nrm_pool.tile([128, 1024], BF16, tag="un")
                nc.vector.tensor_copy(un[0:65, :], acc[0:65, :])
                if len(pending_norm) >= 2:
                    pn_msb, pn_p, pn_emit, pn_outT = pending_norm.popleft()
                    pn_emit()
                    if pn_p == 3:
                        for mb in range(4):
                            for db in range(4):
                                wo_queue.append(
                                    make_wo_group(pn_msb, pn_outT, mb, db))
                r16 = nrm_pool.tile([1, 1024], BF16, tag="r16")
                with nc.allow_low_precision(reason="softmax denom recip"):
                    nc.vector.reciprocal(r16[:, 0:512], un[64:65, 0:512])
                    nc.vector.reciprocal(r16[:, 512:1024], un[64:65, 512:1024])
                pending_norm.append(
                    (msb, p, make_normalize(un, r16, outT_t, p), outT_t))
        while pending_norm:
            pn_msb, pn_p, pn_emit, pn_outT = pending_norm.popleft()
            pn_emit()
            if pn_p == 3:
                for mb in range(4):
                    for db in range(4):
                        wo_queue.append(make_wo_group(pn_msb, pn_outT, mb, db))
        while wo_queue:
            wo_queue.popleft()()


_NC_CACHE = {}


def get_nc():
    if "nc" not in _NC_CACHE:
        _NC_CACHE["nc"] = build_bass()
    return _NC_CACHE["nc"]


def kernel(inputs_q, inputs_k, inputs_v, Wq, bq, Wk, bk, Wv, bv, Wo, bo):
    inputs_q = np.asarray(inputs_q, np.float32)
    inputs_k = np.asarray(inputs_k, np.float32)
    inputs_v = np.asarray(inputs_v, np.float32)
    Wq = np.asarray(Wq, np.float32)
    Wk = np.asarray(Wk, np.float32)
    Wv = np.asarray(Wv, np.float32)
    Wo = np.asarray(Wo, np.float32)
    bq = np.asarray(bq, np.float32)
    bk = np.asarray(bk, np.float32)
    bv = np.asarray(bv, np.float32)
    bo = np.asarray(bo, np.float32)

    nc = get_nc()
    trimask = np.triu(np.ones((128, 128), np.float32)).astype(NPBF16)
    identity = np.eye(128, dtype=np.float32).astype(NPBF16)

    def blocked(x):
        # [S, D] -> [kc, nsb, 128 (d), 512 (n)] bf16, each chunk contiguous
        return np.ascontiguousarray(
            x.reshape(4, 512, KC, 128).transpose(2, 0, 3, 1).astype(NPBF16)
        )

    xT = {}
    for b in range(B):
        xT[("q", b)] = blocked(inputs_q[b])
        xT[("k", b)] = blocked(inputs_k[b])
        xT[("v", b)] = blocked(inputs_v[b])

    in_maps = []
    for c in range(8):
        b = c // 4
        g0 = 2 * (c % 4)
        g1 = g0 + 1
        # pair-major channel permutation: (head p of g0, head p of g1), p=0..3
        perm = []
        for p in range(HPG):
            perm.extend(range(256 * g0 + 64 * p, 256 * g0 + 64 * p + 64))
            perm.extend(range(256 * g1 + 64 * p, 256 * g1 + 64 * p + 64))
        perm = np.array(perm)
        in_maps.append({
            "xqT": xT[("q", b)],
            "xkT": xT[("k", b)],
            "xvT": xT[("v", b)],
            "wq": Wq[:, perm].astype(NPBF16),
            "wk": Wk[:, 64 * g0: 64 * g0 + 128].astype(NPBF16),
            "wv": Wv[:, 64 * g0: 64 * g0 + 128].astype(NPBF16),
            "wo": Wo[perm, :].astype(NPBF16),
            "bq": np.ascontiguousarray(bq[perm].reshape(CPC, 1)),
            "bk": np.ascontiguousarray(bk[64 * g0: 64 * g0 + 128].reshape(128, 1)),
            "trimask": trimask,
            "ident": identity,
        })

    res = run_bass_kernel_spmd(nc, in_maps, list(range(8)))

    # bv passes through (attention rows sum to 1): out += bv_expand @ Wo + bo
    bv_expand = np.repeat(bv.reshape(NKV, 1, HD), HPG, axis=1).reshape(D)
    corr = (bv_expand.astype(np.float64) @ Wo.astype(np.float64)) + bo

    outp = np.zeros((B, S, D), np.float32)
    for c in range(8):
        outp[c // 4] += res.results[c]["out"].astype(np.float32)
    outp += corr.astype(np.float32)
    return outp


# revision 31
# speedup vs baseline: 1.1409x; 1.0360x over previous
"""GroupedQueryAttention Trainium2 kernel (8-core SPMD, bf16 datapath).

Problem: B=2, S=2048, D=2048, 32 Q heads, 8 KV groups, head_dim=64.
  q = xq @ Wq + bq; k = xk @ Wk + bk; v = xv @ Wv + bv
  logits = q . k / sqrt(512), causal softmax, out = (attn @ v) @ Wo + bo

Sharding: one batch x two KV groups per core (2 batches x 4 group-pairs = 8).
Each core computes its 8 Q heads' attention and a partial output projection
(rows of Wo for its 512 channels); the host sums the 4 partials per batch and
adds the bv/bo corrections (bv passes through softmax linearly since attention
weights sum to 1, so bv_expand @ Wo + bo is exact).

Perf notes vs the fp32 version:
- All matmul operands are bf16 (host casts inputs/weights; PSUM accumulation
  stays fp32): 1 PE cycle/row at any width vs fp32's two-pass LOW_HIGH mode.
- Softmax denominators are folded into the attn@v matmul via a ones column
  appended to each head-group's V block (M=65), killing the dedicated
  ones-vector matmul streams.
- V is projected transposed (weights stationary, x streaming) then flipped
  with PE transpose ops - much cheaper than streaming 128-wide W with x tiles
  as stationary weights.
- The two logit halves of a key block land in one 2-bank PSUM tile so a single
  wide activation does exp for both (fewer Act fixed overheads).
- Reciprocal uses the fast-approx DVE op (f32), downcast to bf16 on the Act
  engine (single-partition DVE ops are lane-serial and slow).
- Input/output DMA spread across scalar/gpsimd/sync queues (~95GB/s each).
- Wo projection of superblock i is emitted after the first head-pair of
  superblock i+1 so its matmuls never head-block the PE queue.
"""

import math
import numpy as np
import ml_dtypes

import concourse.bass as bass
import concourse.mybir as mybir
from concourse import tile
from concourse.bass_utils import run_bass_kernel_spmd
from concourse.vector_clock import ScopedClock

F32 = mybir.dt.float32
BF16 = mybir.dt.bfloat16
NPBF16 = ml_dtypes.bfloat16
B, S, D = 2, 2048, 2048
NKV, HPG, HD = 8, 4, 64
DIMK = 512                 # k/v projection width; also the softmax scale base
CPC = 512                  # q channels per core (2 groups * 4 heads * 64)
KC = D // 128              # 16 k-chunks
MSB = S // 512             # 4 m-superblocks
NB = S // 128              # 16 n-blocks
VST = 130                  # v_sb per-block stride: 64 v_a | 1 | 64 v_b | 1
INV_SQRT_DIMK = 1.0 / math.sqrt(float(DIMK))


# ---------------------------------------------------------------------------
# TileContext tail-drain patch: the bundled neuronxcc walrus rejects
# instructions carrying more than ~2 sync waits ("Too many sync wait
# commands"). Spread the kernel-tail waits over single-wait nops.
def _patched_drain_and_barrier(self, tick_clock, wait_clock):
    nc = self.nc
    collector = nc.sync.nop(nofuse=True)
    wait_clock.add_sem_waits(
        collector.ins, ScopedClock({None: tick_clock.global_clock})
    )
    si = collector.ins.sync_info
    waits = list(si.on_wait) if si is not None and si.on_wait else []
    if waits:
        collector.ins.sync_info = mybir.SyncInfo(
            on_wait=[waits[0]], on_update=list(si.on_update or [])
        )
        for w in waits[1:]:
            extra = nc.sync.nop(nofuse=True)
            extra.ins.sync_info = mybir.SyncInfo(on_wait=[w], on_update=[])
    nc.sync.drain()
    nc.all_engine_barrier()
    assert self.sems is not None
    popped = nc._tile_sem_poison_stack.pop()
    assert popped is self._sem_poison
    nc.clear_and_free_semaphores(list(self.sems.allocated().values()))
    nc.all_engine_barrier()


tile.TileContext._drain_and_barrier = _patched_drain_and_barrier


_MAXW = 1
_NOPID = [0]


def split_excess_waits(nc):
    """Walrus here encodes at most ~1-2 sync waits per instruction; move the
    excess onto preceding same-engine nops (engine order preserves timing)."""
    for f in nc.m.functions:
        for bb in f.blocks:
            out_list = []
            changed = False
            for inst in bb.instructions:
                si = getattr(inst, "sync_info", None)
                waits = list(si.on_wait) if si is not None and si.on_wait else []
                if len(waits) > _MAXW:
                    changed = True
                    for w in waits[:-_MAXW]:
                        _NOPID[0] += 1
                        nop = mybir.InstNoOp(
                            name=f"waitnop-{_NOPID[0]}", ins=[], outs=[],
                            engine=inst.engine,
                        )
                        nop.sync_info = mybir.SyncInfo(on_wait=[w], on_update=[])
                        out_list.append(nop)
                    inst.sync_info = mybir.SyncInfo(
                        on_wait=waits[-_MAXW:], on_update=list(si.on_update or [])
                    )
                out_list.append(inst)
            if changed:
                bb.instructions[:] = out_list
# ---------------------------------------------------------------------------


def build_bass():
    nc = bass.Bass()
    # x inputs arrive pre-blocked [nsb, kc-pair, 128, 1024]: each partition's
    # line holds two adjacent kc chunks' 512 values contiguously, so every
    # chunk DMA is one fully linear 256KB read with 2KB per-partition lines
    # (1KB lines measured ~64GB/s per queue; 2KB ~95GB/s).
    xqT = nc.dram_tensor("xqT", [4, KC // 2, 128, 1024], BF16, kind="ExternalInput")
    xkT = nc.dram_tensor("xkT", [4, KC // 2, 128, 1024], BF16, kind="ExternalInput")
    xvT = nc.dram_tensor("xvT", [4, KC // 2, 128, 1024], BF16, kind="ExternalInput")
    wq = nc.dram_tensor("wq", [D, CPC], BF16, kind="ExternalInput")
    wk = nc.dram_tensor("wk", [D, 128], BF16, kind="ExternalInput")
    wv = nc.dram_tensor("wv", [D, 128], BF16, kind="ExternalInput")
    wo = nc.dram_tensor("wo", [CPC, D], BF16, kind="ExternalInput")
    bq = nc.dram_tensor("bq", [CPC, 1], F32, kind="ExternalInput")
    bk = nc.dram_tensor("bk", [128, 1], F32, kind="ExternalInput")
    trimask = nc.dram_tensor("trimask", [128, 128], BF16, kind="ExternalInput")
    ident = nc.dram_tensor("ident", [128, 128], BF16, kind="ExternalInput")
    out = nc.dram_tensor("out", [S, D], BF16, kind="ExternalOutput")

    from contextlib import ExitStack
    with tile.TileContext(nc) as tc, ExitStack() as ctx:
        build_body(ctx, tc, xqT, xkT, xvT, wq, wk, wv, wo, bq, bk,
                   trimask, ident, out)
    split_excess_waits(nc)
    return nc


def build_body(ctx, tc, xqT, xkT, xvT, wq, wk, wv, wo, bq, bk,
               trimask, ident, out):
    nc = tc.nc
    Exp = mybir.ActivationFunctionType.Exp
    Ident = mybir.ActivationFunctionType.Identity
    Copy = mybir.ActivationFunctionType.Copy

    const = ctx.enter_context(tc.tile_pool(name="const", bufs=1))
    wq_sb = const.tile([128, KC * CPC], BF16, tag="wq")      # [128, 8192]
    wk_sb = const.tile([128, KC * 128], BF16, tag="wk")      # [128, 2048]
    wv_sb = const.tile([128, KC * 128], BF16, tag="wv")      # [128, 2048]
    wo_sb = const.tile([128, 4 * D], BF16, tag="wo")         # [128, 8192]
    kT_sb = const.tile([128, S], BF16, tag="kT")             # [128, 2048]
    v_sb = const.tile([128, NB * VST], BF16, tag="v")        # [128, 2080]
    qT_sb = const.tile([128, 4 * S], BF16, tag="qT")         # [128, 8192]
    bq_sb = const.tile([128, 4], F32, tag="bq")
    bk_sb = const.tile([128, 1], F32, tag="bk")
    mask_sb = const.tile([128, 128], BF16, tag="mask")
    ident_sb = const.tile([128, 128], BF16, tag="ident")
    ones_row = const.tile([1, 64], BF16, tag="ones_row")

    # Weight / bias / mask loads: each weight rides its consumer stream's
    # queue just ahead of the x chunks (xk->gpsimd, xv->sync, xq->scalar);
    # wo is loaded late (emitted after the projection loop) on sync.
    nc.gpsimd.dma_start(
        wk_sb[:].rearrange("p (kc c) -> p kc c", kc=KC),
        wk.rearrange("(kc p) c -> p kc c", p=128),
    )
    nc.sync.dma_start(
        wv_sb[:].rearrange("p (kc c) -> p kc c", kc=KC),
        wv.rearrange("(kc p) c -> p kc c", p=128),
    )
    nc.scalar.dma_start(
        wq_sb[:].rearrange("p (kc c) -> p kc c", kc=KC),
        wq.rearrange("(kc p) c -> p kc c", p=128),
    )
    nc.scalar.dma_start(
        bq_sb[:].rearrange("p (cb o) -> p cb o", cb=4),
        bq.rearrange("(cb p) o -> p cb o", p=128),
    )
    nc.scalar.dma_start(bk_sb[:], bk[:])
    nc.scalar.dma_start(mask_sb[:], trimask[:])
    nc.scalar.dma_start(ident_sb[:], ident[:])
    nc.vector.memset(v_sb[:], 1.0)   # ones columns at 64/129 of each block
    nc.vector.memset(ones_row[:], 1.0)

    # ---------------- Phase 1-3: projections ----------------
    with tc.tile_pool(name="proj_psum", bufs=6, space="PSUM") as proj_psum, \
         tc.tile_pool(name="tp_psum", bufs=2, space="PSUM") as tp_psum, \
         tc.tile_pool(name="xin", bufs=5) as xin_pool, \
         tc.tile_pool(name="xvin", bufs=5) as xvin_pool, \
         tc.tile_pool(name="vt", bufs=2) as vt_pool:

        # K/V/Q projections interleaved per 512-token superblock so that all
        # three x input streams (gpsimd/sync/scalar queues) run concurrently.
        for nsb in range(4):
            # K: kT[c=128, n] accumulated over k-chunks, bias bk.
            ps = proj_psum.tile([128, 512], F32, tag="ps")
            for kp in range(KC // 2):
                xk_t = xin_pool.tile([128, 1024], BF16, tag="xk")
                nc.gpsimd.dma_start(xk_t[:], xkT[nsb, kp])
                for h in range(2):
                    kc = 2 * kp + h
                    nc.tensor.matmul(
                        ps[:], wk_sb[:, kc * 128:(kc + 1) * 128],
                        xk_t[:, h * 512:(h + 1) * 512],
                        start=(kc == 0), stop=(kc == KC - 1),
                    )
            nc.scalar.activation(
                kT_sb[:, nsb * 512:(nsb + 1) * 512], ps[:], Ident, bias=bk_sb[:]
            )

            # V: projected transposed (vT[c, n]) with wv stationary, then PE
            # transposes into v natural layout with interleaved ones columns.
            ps = proj_psum.tile([128, 512], F32, tag="ps")
            for kp in range(KC // 2):
                xv_t = xvin_pool.tile([128, 1024], BF16, tag="xv")
                nc.sync.dma_start(xv_t[:], xvT[nsb, kp])
                for h in range(2):
                    kc = 2 * kp + h
                    nc.tensor.matmul(
                        ps[:], wv_sb[:, kc * 128:(kc + 1) * 128],
                        xv_t[:, h * 512:(h + 1) * 512],
                        start=(kc == 0), stop=(kc == KC - 1),
                    )
            vT_t = vt_pool.tile([128, 512], BF16, tag="vt")
            nc.vector.tensor_copy(vT_t[:], ps[:])
            for t in range(4):
                j = nsb * 4 + t
                tp = tp_psum.tile([128, 128], BF16, tag="tp")
                nc.tensor.transpose(tp[:], vT_t[:, t * 128:(t + 1) * 128],
                                    ident_sb[:])
                nc.vector.tensor_copy(v_sb[:, j * VST: j * VST + 64],
                                      tp[:, 0:64])
                nc.vector.tensor_copy(v_sb[:, j * VST + 65: j * VST + 129],
                                      tp[:, 64:128])

            # Q: qT[c, m], layout [msb][cb][512], bias bq (msb == nsb).
            msb = nsb
            pss = [proj_psum.tile([128, 512], F32, tag="ps", name=f"psq{cb}")
                   for cb in range(4)]
            for kp in range(KC // 2):
                xq_t = xin_pool.tile([128, 1024], BF16, tag="xq")
                nc.scalar.dma_start(xq_t[:], xqT[msb, kp])
                for h in range(2):
                    kc = 2 * kp + h
                    for cb in range(4):
                        nc.tensor.matmul(
                            pss[cb][:],
                            wq_sb[:, kc * CPC + cb * 128: kc * CPC + (cb + 1) * 128],
                            xq_t[:, h * 512:(h + 1) * 512],
                            start=(kc == 0), stop=(kc == KC - 1),
                        )
            for cb in range(4):
                nc.scalar.activation(
                    qT_sb[:, msb * 2048 + cb * 512: msb * 2048 + (cb + 1) * 512],
                    pss[cb][:], Ident, bias=bq_sb[:, cb:cb + 1],
                )

    # wo lands during early attention; first consumer is ~2/3 in.
    nc.sync.dma_start(
        wo_sb[:].rearrange("p (cb d) -> p cb d", cb=4),
        wo.rearrange("(cb p) d -> p cb d", p=128),
    )

    # ---------------- Phase 4: attention + output projection ----------------
    with tc.tile_pool(name="lt_psum", bufs=2, space="PSUM") as lt_psum, \
         tc.tile_pool(name="acc_psum", bufs=1, space="PSUM") as acc_psum, \
         tc.tile_pool(name="aux_psum", bufs=2, space="PSUM") as aux_psum, \
         tc.tile_pool(name="pt", bufs=3) as pt_pool, \
         tc.tile_pool(name="outT", bufs=3) as outT_pool, \
         tc.tile_pool(name="nrm", bufs=3) as nrm_pool, \
         tc.tile_pool(name="osb", bufs=3) as out_pool:

        from collections import deque

        def make_wo_group(msb, outT_t, mb, db):
            def emit():
                pso = aux_psum.tile([128, 512], F32, tag="aux")
                for cb in range(4):
                    nc.tensor.matmul(
                        pso[:],
                        outT_t[:, cb * 512 + mb * 128: cb * 512 + (mb + 1) * 128],
                        wo_sb[:, cb * D + db * 512: cb * D + (db + 1) * 512],
                        start=(cb == 0), stop=(cb == 3),
                    )
                o_t = out_pool.tile([128, 512], BF16, tag="osb")
                nc.vector.tensor_copy(o_t[:], pso[:])
                nc.sync.dma_start(
                    out[msb * 512 + mb * 128: msb * 512 + (mb + 1) * 128,
                        db * 512:(db + 1) * 512],
                    o_t[:],
                )
            return emit

        def make_normalize(un, r16, outT_t, p):
            # Deferred tail of the normalize: PE outer-product broadcast of
            # the reciprocal row, fused multiply into outT. The reciprocal
            # itself was issued eagerly (it is a ~5us single-lane DVE op that
            # must complete during the next head-pair's j-loop, not block it).
            def emit():
                bps = aux_psum.tile([128, 512], F32, tag="aux")
                nc.tensor.matmul(
                    bps[0:64, :], ones_row[:], r16[:, 0:512],
                    start=True, stop=True, tile_position=(0, 0),
                )
                nc.tensor.matmul(
                    bps[64:128, :], ones_row[:], r16[:, 512:1024],
                    start=True, stop=True, tile_position=(0, 64),
                )
                bc = nrm_pool.tile([64, 1024], BF16, tag="bc")
                nc.vector.tensor_copy(bc[:, 0:512], bps[0:64, :])
                nc.vector.tensor_copy(bc[:, 512:1024], bps[64:128, :])
                nc.vector.tensor_mul(
                    outT_t[0:64, p * 512:(p + 1) * 512], un[0:64, 0:512],
                    bc[:, 0:512],
                )
                nc.vector.tensor_mul(
                    outT_t[64:128, p * 512:(p + 1) * 512], un[0:64, 512:1024],
                    bc[:, 512:1024],
                )
            return emit

        # Deferred-work plumbing: the normalize chain for head-pair p is
        # emitted during p+1's j-loop so its DVE/PE ops never head-block the
        # PE queue, and Wo matmul groups drip in two key-blocks apart so the
        # exp pipeline never drains during an output-projection burst.
        pending_norm = deque()
        wo_queue = deque()
        js_since_wo = [0]

        def drip_wo():
            js_since_wo[0] += 1
            if wo_queue and js_since_wo[0] >= 2:
                wo_queue.popleft()()
                js_since_wo[0] = 0

        for msb in range(MSB):
            outT_t = outT_pool.tile([128, 2048], BF16, tag="outT")
            for p in range(4):
                acc = acc_psum.tile([128, 1024], F32, tag="acc")
                njb = 4 * msb + 4
                qbase = msb * 2048 + p * 512
                for j in range(njb):
                    if j < 4 * msb:
                        moff, W = 0, 512
                    else:
                        t = j - 4 * msb
                        moff, W = 128 * t, 512 - 128 * t
                    first = (j == 0)
                    last = (j == njb - 1)
                    qlo = qT_sb[0:64, qbase + moff: qbase + moff + W]
                    qhi = qT_sb[64:128, qbase + moff: qbase + moff + W]
                    lt = lt_psum.tile([128, 1024], F32, tag="lt")
                    nc.tensor.matmul(
                        lt[:, 0:W],
                        kT_sb[0:64, j * 128:(j + 1) * 128], qlo,
                        start=True, stop=True, tile_position=(0, 0),
                    )
                    nc.tensor.matmul(
                        lt[:, 512:512 + W],
                        kT_sb[64:128, j * 128:(j + 1) * 128], qhi,
                        start=True, stop=True, tile_position=(64, 0),
                    )
                    pt = pt_pool.tile([128, 1024], BF16, tag="pt")
                    nc.scalar.activation(pt[:], lt[:], Exp, scale=INV_SQRT_DIMK)
                    if j >= 4 * msb:  # diagonal: mask the leading triangle
                        nc.gpsimd.tensor_mul(pt[:, 0:128], pt[:, 0:128],
                                             mask_sb[:])
                        nc.gpsimd.tensor_mul(pt[:, 512:640], pt[:, 512:640],
                                             mask_sb[:])
                    # attn @ v with the denominator folded in (ones col at 64)
                    nc.tensor.matmul(
                        acc[0:65, moff:moff + W],
                        v_sb[:, j * VST: j * VST + 65], pt[:, 0:W],
                        start=first, stop=last,
                    )
                    nc.tensor.matmul(
                        acc[0:65, 512 + moff:512 + moff + W],
                        v_sb[:, j * VST + 65: j * VST + 130], pt[:, 512:512 + W],
                        start=first, stop=last,
                    )
                    drip_wo()
                # DVE order at a pair boundary: (1) evict acc so its single
                # PSUM buffer frees ASAP, (2) flush the deferred normalize of
                # the previous pair (its reciprocal finished during this
                # j-loop), (3) start this pair's slow single-lane reciprocal
                # last so it hides under the next j-loop.
                un = nrm_pool.tile([128, 1024], BF16, tag="un")
                nc.vector.tensor_copy(un[0:65, :], acc[0:65, :])
                if len(pending_norm) >= 2:
                    pn_msb, pn_p, pn_emit, pn_outT = pending_norm.popleft()
                    pn_emit()
                    if pn_p == 3:
                        for mb in range(4):
                            for db in range(4):
                                wo_queue.append(
                                    make_wo_group(pn_msb, pn_outT, mb, db))
                r16 = nrm_pool.tile([1, 1024], BF16, tag="r16")
                with nc.allow_low_precision(reason="softmax denom recip"):
                    nc.vector.reciprocal(r16[:, 0:512], un[64:65, 0:512])
                    nc.vector.reciprocal(r16[:, 512:1024], un[64:65, 512:1024])
                pending_norm.append(
                    (msb, p, make_normalize(un, r16, outT_t, p), outT_t))
        while pending_norm:
            pn_msb, pn_p, pn_emit, pn_outT = pending_norm.popleft()
            pn_emit()
            if pn_p == 3:
                for mb in range(4):
                    for db in range(4):
                        wo_queue.append(make_wo_group(pn_msb, pn_outT, mb, db))
        while wo_queue:
            wo_queue.popleft()()


_NC_CACHE = {}


def get_nc():
    if "nc" not in _NC_CACHE:
        _NC_CACHE["nc"] = build_bass()
    return _NC_CACHE["nc"]


def kernel(inputs_q, inputs_k, inputs_v, Wq, bq, Wk, bk, Wv, bv, Wo, bo):
    inputs_q = np.asarray(inputs_q, np.float32)
    inputs_k = np.asarray(inputs_k, np.float32)
    inputs_v = np.asarray(inputs_v, np.float32)
    Wq = np.asarray(Wq, np.float32)
    Wk = np.asarray(Wk, np.float32)
    Wv = np.asarray(Wv, np.float32)
    Wo = np.asarray(Wo, np.float32)
    bq = np.asarray(bq, np.float32)
    bk = np.asarray(bk, np.float32)
    bv = np.asarray(bv, np.float32)
    bo = np.asarray(bo, np.float32)

    nc = get_nc()
    trimask = np.triu(np.ones((128, 128), np.float32)).astype(NPBF16)
    identity = np.eye(128, dtype=np.float32).astype(NPBF16)

    def blocked(x):
        # [S, D] -> [nsb, kc-pair, 128 (d), 1024 (2 x 512 n)] bf16: each
        # partition's line packs two adjacent kc chunks contiguously.
        a = x.reshape(4, 512, KC, 128).transpose(0, 2, 3, 1)  # [nsb,kc,p,n]
        a = a.reshape(4, KC // 2, 2, 128, 512).transpose(0, 1, 3, 2, 4)
        return np.ascontiguousarray(a.reshape(4, KC // 2, 128, 1024)
                                    .astype(NPBF16))

    xT = {}
    for b in range(B):
        xT[("q", b)] = blocked(inputs_q[b])
        xT[("k", b)] = blocked(inputs_k[b])
        xT[("v", b)] = blocked(inputs_v[b])

    in_maps = []
    for c in range(8):
        b = c // 4
        g0 = 2 * (c % 4)
        g1 = g0 + 1
        # pair-major channel permutation: (head p of g0, head p of g1), p=0..3
        perm = []
        for p in range(HPG):
            perm.extend(range(256 * g0 + 64 * p, 256 * g0 + 64 * p + 64))
            perm.extend(range(256 * g1 + 64 * p, 256 * g1 + 64 * p + 64))
        perm = np.array(perm)
        in_maps.append({
            "xqT": xT[("q", b)],
            "xkT": xT[("k", b)],
            "xvT": xT[("v", b)],
            "wq": Wq[:, perm].astype(NPBF16),
            "wk": Wk[:, 64 * g0: 64 * g0 + 128].astype(NPBF16),
            "wv": Wv[:, 64 * g0: 64 * g0 + 128].astype(NPBF16),
            "wo": Wo[perm, :].astype(NPBF16),
            "bq": np.ascontiguousarray(bq[perm].reshape(CPC, 1)),
            "bk": np.ascontiguousarray(bk[64 * g0: 64 * g0 + 128].reshape(128, 1)),
            "trimask": trimask,
            "ident": identity,
        })

    res = run_bass_kernel_spmd(nc, in_maps, list(range(8)))

    # bv passes through (attention rows sum to 1): out += bv_expand @ Wo + bo
    bv_expand = np.repeat(bv.reshape(NKV, 1, HD), HPG, axis=1).reshape(D)
    corr = (bv_expand.astype(np.float64) @ Wo.astype(np.float64)) + bo

    outp = np.zeros((B, S, D), np.float32)
    for c in range(8):
        outp[c // 4] += res.results[c]["out"].astype(np.float32)
    outp += corr.astype(np.float32)
    return outp


# revision 32
# speedup vs baseline: 1.1595x; 1.0163x over previous
"""GroupedQueryAttention Trainium2 kernel (8-core SPMD, bf16 datapath).

Problem: B=2, S=2048, D=2048, 32 Q heads, 8 KV groups, head_dim=64.
  q = xq @ Wq + bq; k = xk @ Wk + bk; v = xv @ Wv + bv
  logits = q . k / sqrt(512), causal softmax, out = (attn @ v) @ Wo + bo

Sharding: one batch x two KV groups per core (2 batches x 4 group-pairs = 8).
Each core computes its 8 Q heads' attention and a partial output projection
(rows of Wo for its 512 channels); the host sums the 4 partials per batch and
adds the bv/bo corrections (bv passes through softmax linearly since attention
weights sum to 1, so bv_expand @ Wo + bo is exact).

Perf notes vs the fp32 version:
- All matmul operands are bf16 (host casts inputs/weights; PSUM accumulation
  stays fp32): 1 PE cycle/row at any width vs fp32's two-pass LOW_HIGH mode.
- Softmax denominators are folded into the attn@v matmul via a ones column
  appended to each head-group's V block (M=65), killing the dedicated
  ones-vector matmul streams.
- V is projected transposed (weights stationary, x streaming) then flipped
  with PE transpose ops - much cheaper than streaming 128-wide W with x tiles
  as stationary weights.
- The two logit halves of a key block land in one 2-bank PSUM tile so a single
  wide activation does exp for both (fewer Act fixed overheads).
- Reciprocal uses the fast-approx DVE op (f32), downcast to bf16 on the Act
  engine (single-partition DVE ops are lane-serial and slow).
- Input/output DMA spread across scalar/gpsimd/sync queues (~95GB/s each).
- Wo projection of superblock i is emitted after the first head-pair of
  superblock i+1 so its matmuls never head-block the PE queue.
"""

import math
import numpy as np
import ml_dtypes

import concourse.bass as bass
import concourse.mybir as mybir
from concourse import tile
from concourse.bass_utils import run_bass_kernel_spmd
from concourse.vector_clock import ScopedClock

F32 = mybir.dt.float32
BF16 = mybir.dt.bfloat16
NPBF16 = ml_dtypes.bfloat16
B, S, D = 2, 2048, 2048
NKV, HPG, HD = 8, 4, 64
DIMK = 512                 # k/v projection width; also the softmax scale base
CPC = 512                  # q channels per core (2 groups * 4 heads * 64)
KC = D // 128              # 16 k-chunks
MSB = S // 512             # 4 m-superblocks
NB = S // 128              # 16 n-blocks
VST = 130                  # v_sb per-block stride: 64 v_a | 1 | 64 v_b | 1
INV_SQRT_DIMK = 1.0 / math.sqrt(float(DIMK))


# ---------------------------------------------------------------------------
# TileContext tail-drain patch: the bundled neuronxcc walrus rejects
# instructions carrying more than ~2 sync waits ("Too many sync wait
# commands"). Spread the kernel-tail waits over single-wait nops.
def _patched_drain_and_barrier(self, tick_clock, wait_clock):
    nc = self.nc
    collector = nc.sync.nop(nofuse=True)
    wait_clock.add_sem_waits(
        collector.ins, ScopedClock({None: tick_clock.global_clock})
    )
    si = collector.ins.sync_info
    waits = list(si.on_wait) if si is not None and si.on_wait else []
    if waits:
        collector.ins.sync_info = mybir.SyncInfo(
            on_wait=[waits[0]], on_update=list(si.on_update or [])
        )
        for w in waits[1:]:
            extra = nc.sync.nop(nofuse=True)
            extra.ins.sync_info = mybir.SyncInfo(on_wait=[w], on_update=[])
    nc.sync.drain()
    nc.all_engine_barrier()
    assert self.sems is not None
    popped = nc._tile_sem_poison_stack.pop()
    assert popped is self._sem_poison
    nc.clear_and_free_semaphores(list(self.sems.allocated().values()))
    nc.all_engine_barrier()


tile.TileContext._drain_and_barrier = _patched_drain_and_barrier


_MAXW = 1
_NOPID = [0]


def split_excess_waits(nc):
    """Walrus here encodes at most ~1-2 sync waits per instruction; move the
    excess onto preceding same-engine nops (engine order preserves timing)."""
    for f in nc.m.functions:
        for bb in f.blocks:
            out_list = []
            changed = False
            for inst in bb.instructions:
                si = getattr(inst, "sync_info", None)
                waits = list(si.on_wait) if si is not None and si.on_wait else []
                if len(waits) > _MAXW:
                    changed = True
                    for w in waits[:-_MAXW]:
                        _NOPID[0] += 1
                        nop = mybir.InstNoOp(
                            name=f"waitnop-{_NOPID[0]}", ins=[], outs=[],
                            engine=inst.engine,
                        )
                        nop.sync_info = mybir.SyncInfo(on_wait=[w], on_update=[])
                        out_list.append(nop)
                    inst.sync_info = mybir.SyncInfo(
                        on_wait=waits[-_MAXW:], on_update=list(si.on_update or [])
                    )
                out_list.append(inst)
            if changed:
                bb.instructions[:] = out_list
# ---------------------------------------------------------------------------


def build_bass():
    nc = bass.Bass()
    # x inputs arrive pre-blocked [nsb, kc-pair, 128, 1024]: each partition's
    # line holds two adjacent kc chunks' 512 values contiguously, so every
    # chunk DMA is one fully linear 256KB read with 2KB per-partition lines
    # (1KB lines measured ~64GB/s per queue; 2KB ~95GB/s).
    xqT = nc.dram_tensor("xqT", [4, KC // 2, 128, 1024], BF16, kind="ExternalInput")
    xkT = nc.dram_tensor("xkT", [4, KC // 2, 128, 1024], BF16, kind="ExternalInput")
    xvT = nc.dram_tensor("xvT", [4, KC // 2, 128, 1024], BF16, kind="ExternalInput")
    wq = nc.dram_tensor("wq", [D, CPC], BF16, kind="ExternalInput")
    wk = nc.dram_tensor("wk", [D, 128], BF16, kind="ExternalInput")
    wv = nc.dram_tensor("wv", [D, 128], BF16, kind="ExternalInput")
    wo = nc.dram_tensor("wo", [CPC, D], BF16, kind="ExternalInput")
    bq = nc.dram_tensor("bq", [CPC, 1], F32, kind="ExternalInput")
    bk = nc.dram_tensor("bk", [128, 1], F32, kind="ExternalInput")
    trimask = nc.dram_tensor("trimask", [128, 128], BF16, kind="ExternalInput")
    ident = nc.dram_tensor("ident", [128, 128], BF16, kind="ExternalInput")
    out = nc.dram_tensor("out", [S, D], BF16, kind="ExternalOutput")

    from contextlib import ExitStack
    with tile.TileContext(nc) as tc, ExitStack() as ctx:
        build_body(ctx, tc, xqT, xkT, xvT, wq, wk, wv, wo, bq, bk,
                   trimask, ident, out)
    split_excess_waits(nc)
    return nc


def build_body(ctx, tc, xqT, xkT, xvT, wq, wk, wv, wo, bq, bk,
               trimask, ident, out):
    nc = tc.nc
    Exp = mybir.ActivationFunctionType.Exp
    Ident = mybir.ActivationFunctionType.Identity
    Copy = mybir.ActivationFunctionType.Copy

    const = ctx.enter_context(tc.tile_pool(name="const", bufs=1))
    wq_sb = const.tile([128, KC * CPC], BF16, tag="wq")      # [128, 8192]
    wk_sb = const.tile([128, KC * 128], BF16, tag="wk")      # [128, 2048]
    wv_sb = const.tile([128, KC * 128], BF16, tag="wv")      # [128, 2048]
    wo_sb = const.tile([128, 4 * D], BF16, tag="wo")         # [128, 8192]
    kT_sb = const.tile([128, S], BF16, tag="kT")             # [128, 2048]
    v_sb = const.tile([128, NB * VST], BF16, tag="v")        # [128, 2080]
    qT_sb = const.tile([128, 4 * S], BF16, tag="qT")         # [128, 8192]
    bq_sb = const.tile([128, 4], F32, tag="bq")
    bk_sb = const.tile([128, 1], F32, tag="bk")
    mask_sb = const.tile([128, 128], BF16, tag="mask")
    ident_sb = const.tile([128, 128], BF16, tag="ident")
    ones_row = const.tile([1, 64], BF16, tag="ones_row")

    # Weight / bias / mask loads: each weight rides its consumer stream's
    # queue just ahead of the x chunks (xk->gpsimd, xv->sync, xq->scalar);
    # wo is loaded late (emitted after the projection loop) on sync.
    nc.gpsimd.dma_start(
        wk_sb[:].rearrange("p (kc c) -> p kc c", kc=KC),
        wk.rearrange("(kc p) c -> p kc c", p=128),
    )
    nc.sync.dma_start(
        wv_sb[:].rearrange("p (kc c) -> p kc c", kc=KC),
        wv.rearrange("(kc p) c -> p kc c", p=128),
    )
    nc.scalar.dma_start(
        wq_sb[:, 0:KC * CPC // 2].rearrange("p (kc c) -> p kc c", kc=KC // 2),
        wq[0:D // 2].rearrange("(kc p) c -> p kc c", p=128),
    )
    nc.gpsimd.dma_start(
        wq_sb[:, KC * CPC // 2:].rearrange("p (kc c) -> p kc c", kc=KC // 2),
        wq[D // 2:].rearrange("(kc p) c -> p kc c", p=128),
    )
    nc.scalar.dma_start(
        bq_sb[:].rearrange("p (cb o) -> p cb o", cb=4),
        bq.rearrange("(cb p) o -> p cb o", p=128),
    )
    nc.scalar.dma_start(bk_sb[:], bk[:])
    nc.scalar.dma_start(mask_sb[:], trimask[:])
    nc.scalar.dma_start(ident_sb[:], ident[:])
    nc.vector.memset(v_sb[:], 1.0)   # ones columns at 64/129 of each block
    nc.vector.memset(ones_row[:], 1.0)

    # ---------------- Phase 1-3: projections ----------------
    with tc.tile_pool(name="proj_psum", bufs=6, space="PSUM") as proj_psum, \
         tc.tile_pool(name="tp_psum", bufs=2, space="PSUM") as tp_psum, \
         tc.tile_pool(name="xin", bufs=5) as xin_pool, \
         tc.tile_pool(name="xvin", bufs=5) as xvin_pool, \
         tc.tile_pool(name="vt", bufs=2) as vt_pool:

        # K/V/Q projections interleaved per 512-token superblock so that all
        # three x input streams (gpsimd/sync/scalar queues) run concurrently.
        for nsb in range(4):
            # K: kT[c=128, n] accumulated over k-chunks, bias bk.
            ps = proj_psum.tile([128, 512], F32, tag="ps")
            for kp in range(KC // 2):
                xk_t = xin_pool.tile([128, 1024], BF16, tag="xk")
                nc.gpsimd.dma_start(xk_t[:], xkT[nsb, kp])
                for h in range(2):
                    kc = 2 * kp + h
                    nc.tensor.matmul(
                        ps[:], wk_sb[:, kc * 128:(kc + 1) * 128],
                        xk_t[:, h * 512:(h + 1) * 512],
                        start=(kc == 0), stop=(kc == KC - 1),
                    )
            nc.scalar.activation(
                kT_sb[:, nsb * 512:(nsb + 1) * 512], ps[:], Ident, bias=bk_sb[:]
            )

            # V: projected transposed (vT[c, n]) with wv stationary, then PE
            # transposes into v natural layout with interleaved ones columns.
            ps = proj_psum.tile([128, 512], F32, tag="ps")
            for kp in range(KC // 2):
                xv_t = xvin_pool.tile([128, 1024], BF16, tag="xv")
                nc.sync.dma_start(xv_t[:], xvT[nsb, kp])
                for h in range(2):
                    kc = 2 * kp + h
                    nc.tensor.matmul(
                        ps[:], wv_sb[:, kc * 128:(kc + 1) * 128],
                        xv_t[:, h * 512:(h + 1) * 512],
                        start=(kc == 0), stop=(kc == KC - 1),
                    )
            vT_t = vt_pool.tile([128, 512], BF16, tag="vt")
            nc.vector.tensor_copy(vT_t[:], ps[:])
            for t in range(4):
                j = nsb * 4 + t
                tp = tp_psum.tile([128, 128], BF16, tag="tp")
                nc.tensor.transpose(tp[:], vT_t[:, t * 128:(t + 1) * 128],
                                    ident_sb[:])
                nc.vector.tensor_copy(v_sb[:, j * VST: j * VST + 64],
                                      tp[:, 0:64])
                nc.vector.tensor_copy(v_sb[:, j * VST + 65: j * VST + 129],
                                      tp[:, 64:128])

            # Q: qT[c, m], layout [msb][cb][512], bias bq (msb == nsb).
            msb = nsb
            pss = [proj_psum.tile([128, 512], F32, tag="ps", name=f"psq{cb}")
                   for cb in range(4)]
            for kp in range(KC // 2):
                xq_t = xin_pool.tile([128, 1024], BF16, tag="xq")
                nc.scalar.dma_start(xq_t[:], xqT[msb, kp])
                for h in range(2):
                    kc = 2 * kp + h
                    for cb in range(4):
                        nc.tensor.matmul(
                            pss[cb][:],
                            wq_sb[:, kc * CPC + cb * 128: kc * CPC + (cb + 1) * 128],
                            xq_t[:, h * 512:(h + 1) * 512],
                            start=(kc == 0), stop=(kc == KC - 1),
                        )
            for cb in range(4):
                nc.scalar.activation(
                    qT_sb[:, msb * 2048 + cb * 512: msb * 2048 + (cb + 1) * 512],
                    pss[cb][:], Ident, bias=bq_sb[:, cb:cb + 1],
                )

    # wo lands during early attention; first consumer is ~2/3 in.
    nc.sync.dma_start(
        wo_sb[:].rearrange("p (cb d) -> p cb d", cb=4),
        wo.rearrange("(cb p) d -> p cb d", p=128),
    )

    # ---------------- Phase 4: attention + output projection ----------------
    with tc.tile_pool(name="lt_psum", bufs=2, space="PSUM") as lt_psum, \
         tc.tile_pool(name="acc_psum", bufs=1, space="PSUM") as acc_psum, \
         tc.tile_pool(name="aux_psum", bufs=2, space="PSUM") as aux_psum, \
         tc.tile_pool(name="pt", bufs=3) as pt_pool, \
         tc.tile_pool(name="outT", bufs=3) as outT_pool, \
         tc.tile_pool(name="nrm", bufs=3) as nrm_pool, \
         tc.tile_pool(name="osb", bufs=3) as out_pool:

        from collections import deque

        def make_wo_group(msb, outT_t, mb, db):
            def emit():
                pso = aux_psum.tile([128, 512], F32, tag="aux")
                for cb in range(4):
                    nc.tensor.matmul(
                        pso[:],
                        outT_t[:, cb * 512 + mb * 128: cb * 512 + (mb + 1) * 128],
                        wo_sb[:, cb * D + db * 512: cb * D + (db + 1) * 512],
                        start=(cb == 0), stop=(cb == 3),
                    )
                o_t = out_pool.tile([128, 512], BF16, tag="osb")
                nc.vector.tensor_copy(o_t[:], pso[:])
                nc.sync.dma_start(
                    out[msb * 512 + mb * 128: msb * 512 + (mb + 1) * 128,
                        db * 512:(db + 1) * 512],
                    o_t[:],
                )
            return emit

        def make_normalize(un, r16, outT_t, p):
            # Deferred tail of the normalize: PE outer-product broadcast of
            # the reciprocal row, fused multiply into outT. The reciprocal
            # itself was issued eagerly (it is a ~5us single-lane DVE op that
            # must complete during the next head-pair's j-loop, not block it).
            def emit():
                bps = aux_psum.tile([128, 512], F32, tag="aux")
                nc.tensor.matmul(
                    bps[0:64, :], ones_row[:], r16[:, 0:512],
                    start=True, stop=True, tile_position=(0, 0),
                )
                nc.tensor.matmul(
                    bps[64:128, :], ones_row[:], r16[:, 512:1024],
                    start=True, stop=True, tile_position=(0, 64),
                )
                bc = nrm_pool.tile([64, 1024], BF16, tag="bc")
                nc.vector.tensor_copy(bc[:, 0:512], bps[0:64, :])
                nc.vector.tensor_copy(bc[:, 512:1024], bps[64:128, :])
                nc.vector.tensor_mul(
                    outT_t[0:64, p * 512:(p + 1) * 512], un[0:64, 0:512],
                    bc[:, 0:512],
                )
                nc.vector.tensor_mul(
                    outT_t[64:128, p * 512:(p + 1) * 512], un[0:64, 512:1024],
                    bc[:, 512:1024],
                )
            return emit

        # Deferred-work plumbing: the normalize chain for head-pair p is
        # emitted during p+1's j-loop so its DVE/PE ops never head-block the
        # PE queue, and Wo matmul groups drip in two key-blocks apart so the
        # exp pipeline never drains during an output-projection burst.
        pending_norm = deque()
        wo_queue = deque()
        js_since_wo = [0]

        def drip_wo():
            js_since_wo[0] += 1
            if wo_queue and js_since_wo[0] >= 2:
                wo_queue.popleft()()
                js_since_wo[0] = 0

        for msb in range(MSB):
            outT_t = outT_pool.tile([128, 2048], BF16, tag="outT")
            for p in range(4):
                acc = acc_psum.tile([128, 1024], F32, tag="acc")
                njb = 4 * msb + 4
                qbase = msb * 2048 + p * 512
                for j in range(njb):
                    if j < 4 * msb:
                        moff, W = 0, 512
                    else:
                        t = j - 4 * msb
                        moff, W = 128 * t, 512 - 128 * t
                    first = (j == 0)
                    last = (j == njb - 1)
                    qlo = qT_sb[0:64, qbase + moff: qbase + moff + W]
                    qhi = qT_sb[64:128, qbase + moff: qbase + moff + W]
                    lt = lt_psum.tile([128, 1024], F32, tag="lt")
                    nc.tensor.matmul(
                        lt[:, 0:W],
                        kT_sb[0:64, j * 128:(j + 1) * 128], qlo,
                        start=True, stop=True, tile_position=(0, 0),
                    )
                    nc.tensor.matmul(
                        lt[:, 512:512 + W],
                        kT_sb[64:128, j * 128:(j + 1) * 128], qhi,
                        start=True, stop=True, tile_position=(64, 0),
                    )
                    pt = pt_pool.tile([128, 1024], BF16, tag="pt")
                    nc.scalar.activation(pt[:, 0:512 + W], lt[:, 0:512 + W],
                                         Exp, scale=INV_SQRT_DIMK)
                    if j >= 4 * msb:  # diagonal: mask the leading triangle
                        nc.gpsimd.tensor_mul(pt[:, 0:128], pt[:, 0:128],
                                             mask_sb[:])
                        nc.gpsimd.tensor_mul(pt[:, 512:640], pt[:, 512:640],
                                             mask_sb[:])
                    # attn @ v with the denominator folded in (ones col at 64)
                    nc.tensor.matmul(
                        acc[0:65, moff:moff + W],
                        v_sb[:, j * VST: j * VST + 65], pt[:, 0:W],
                        start=first, stop=last,
                    )
                    nc.tensor.matmul(
                        acc[0:65, 512 + moff:512 + moff + W],
                        v_sb[:, j * VST + 65: j * VST + 130], pt[:, 512:512 + W],
                        start=first, stop=last,
                    )
                    drip_wo()
                # DVE order at a pair boundary: (1) evict acc so its single
                # PSUM buffer frees ASAP, (2) flush the deferred normalize of
                # the previous pair (its reciprocal finished during this
                # j-loop), (3) start this pair's slow single-lane reciprocal
                # last so it hides under the next j-loop.
                un = nrm_pool.tile([128, 1024], BF16, tag="un")
                nc.vector.tensor_copy(un[0:65, :], acc[0:65, :])
                if len(pending_norm) >= 2:
                    pn_msb, pn_p, pn_emit, pn_outT = pending_norm.popleft()
                    pn_emit()
                    if pn_p == 3:
                        for mb in range(4):
                            for db in range(4):
                                wo_queue.append(
                                    make_wo_group(pn_msb, pn_outT, mb, db))
                r16 = nrm_pool.tile([1, 1024], BF16, tag="r16")
                with nc.allow_low_precision(reason="softmax denom recip"):
                    nc.vector.reciprocal(r16[:, 0:512], un[64:65, 0:512])
                    nc.vector.reciprocal(r16[:, 512:1024], un[64:65, 512:1024])
                pending_norm.append(
                    (msb, p, make_normalize(un, r16, outT_t, p), outT_t))
        while pending_norm:
            pn_msb, pn_p, pn_emit, pn_outT = pending_norm.popleft()
            pn_emit()
            if pn_p == 3:
                for mb in range(4):
                    for db in range(4):
                        wo_queue.append(make_wo_group(pn_msb, pn_outT, mb, db))
        while wo_queue:
            wo_queue.popleft()()


_NC_CACHE = {}


def get_nc():
    if "nc" not in _NC_CACHE:
        _NC_CACHE["nc"] = build_bass()
    return _NC_CACHE["nc"]


def kernel(inputs_q, inputs_k, inputs_v, Wq, bq, Wk, bk, Wv, bv, Wo, bo):
    inputs_q = np.asarray(inputs_q, np.float32)
    inputs_k = np.asarray(inputs_k, np.float32)
    inputs_v = np.asarray(inputs_v, np.float32)
    Wq = np.asarray(Wq, np.float32)
    Wk = np.asarray(Wk, np.float32)
    Wv = np.asarray(Wv, np.float32)
    Wo = np.asarray(Wo, np.float32)
    bq = np.asarray(bq, np.float32)
    bk = np.asarray(bk, np.float32)
    bv = np.asarray(bv, np.float32)
    bo = np.asarray(bo, np.float32)

    nc = get_nc()
    trimask = np.triu(np.ones((128, 128), np.float32)).astype(NPBF16)
    identity = np.eye(128, dtype=np.float32).astype(NPBF16)

    def blocked(x):
        # [S, D] -> [nsb, kc-pair, 128 (d), 1024 (2 x 512 n)] bf16: each
        # partition's line packs two adjacent kc chunks contiguously.
        a = x.reshape(4, 512, KC, 128).transpose(0, 2, 3, 1)  # [nsb,kc,p,n]
        a = a.reshape(4, KC // 2, 2, 128, 512).transpose(0, 1, 3, 2, 4)
        return np.ascontiguousarray(a.reshape(4, KC // 2, 128, 1024)
                                    .astype(NPBF16))

    xT = {}
    for b in range(B):
        xT[("q", b)] = blocked(inputs_q[b])
        xT[("k", b)] = blocked(inputs_k[b])
        xT[("v", b)] = blocked(inputs_v[b])

    in_maps = []
    for c in range(8):
        b = c // 4
        g0 = 2 * (c % 4)
        g1 = g0 + 1
        # pair-major channel permutation: (head p of g0, head p of g1), p=0..3
        perm = []
        for p in range(HPG):
            perm.extend(range(256 * g0 + 64 * p, 256 * g0 + 64 * p + 64))
            perm.extend(range(256 * g1 + 64 * p, 256 * g1 + 64 * p + 64))
        perm = np.array(perm)
        in_maps.append({
            "xqT": xT[("q", b)],
            "xkT": xT[("k", b)],
            "xvT": xT[("v", b)],
            "wq": Wq[:, perm].astype(NPBF16),
            "wk": Wk[:, 64 * g0: 64 * g0 + 128].astype(NPBF16),
            "wv": Wv[:, 64 * g0: 64 * g0 + 128].astype(NPBF16),
            "wo": Wo[perm, :].astype(NPBF16),
            "bq": np.ascontiguousarray(bq[perm].reshape(CPC, 1)),
            "bk": np.ascontiguousarray(bk[64 * g0: 64 * g0 + 128].reshape(128, 1)),
            "trimask": trimask,
            "ident": identity,
        })

    res = run_bass_kernel_spmd(nc, in_maps, list(range(8)))

    # bv passes through (attention rows sum to 1): out += bv_expand @ Wo + bo
    bv_expand = np.repeat(bv.reshape(NKV, 1, HD), HPG, axis=1).reshape(D)
    corr = (bv_expand.astype(np.float64) @ Wo.astype(np.float64)) + bo

    outp = np.zeros((B, S, D), np.float32)
    for c in range(8):
        outp[c // 4] += res.results[c]["out"].astype(np.float32)
    outp += corr.astype(np.float32)
    return outp
